# revision 1
# baseline (speedup 1.0000x reference)
"""GraphUNet Trainium kernel.

Architecture: 9 GCN convs (in, dn0, dn1, dn2, bottom, up0, up1, up2, out)
with top-k pooling / unpooling and batch-norm+relu between them.

Device does the heavy part of each conv: the edge segment-sum
   y[d] = sum_{e: dst=d} norm_e * x[src_e]   (+ self loop via synthetic edges)
and the feature matmul out = y @ W, via:
   - dma_gather of x rows (fp32) into SBUF tiles of 128 edges
   - "norm-hot" matmul: psum[128c x 16nodes] += x_rows.T @ onehot(dstloc)*norm
   - final matmul streaming yT through W.

Host (numpy) does: sharding/metadata build, top-k pools, edge relabeling,
degree/norm precompute, batch-norm, relu, bias, concat.

Sharding: dst-node ranges across 8 cores; x replicated to all cores
(graph/data parallel per the hint, halo exchange realized as full
replication of the small per-conv feature table).
"""

import os
import sys
import math

import numpy as np

sys.path.insert(0, "/opt/trn_rl_repo")

import concourse.bass as bass  # noqa: E402
import concourse.bacc as bacc  # noqa: E402
import concourse.tile as tile  # noqa: E402
from concourse import mybir  # noqa: E402
from concourse import bass_utils  # noqa: E402

# ---- problem constants (hardcoded per task statement) ----
N0 = 100000
C_IN = 128
H = 128
DEPTH = 3
RATIO = 0.5
EPS = 1e-5
NCORES = 8
BLOCK = 25000  # gather-table block rows (int16 index limit 32768)
GRP = 16  # dst nodes per one-hot group
SG = 32  # groups per stripe (one PSUM bank = SG*GRP*4B = 2KB)
TGN = int(os.environ.get("GNN_TGN", "1024"))  # gather idxs*cin per call

FP = mybir.dt.float32
NPFP = np.float32


# ---------------------------------------------------------------------------
# Bass kernel builder (one conv shape). Cached by shape key.
# ---------------------------------------------------------------------------
_KERNEL_CACHE = {}


def build_conv_kernel(n_rows_pad, cin, n_core_pad, B, C):
    """Kernel for one conv level: per-core dst shard of n_core_pad nodes.

    DRAM inputs (per core):
      xb      fp32 [B*BLOCK, cin]   full (padded) node features
      idxw    int16 [16, T, 8]      wrapped per-tile local src indices
      dstloc  fp32 [128, T]         per-edge dst offset within group (0..15)
      nrm     fp32 [128, T]         per-edge norm (0 for padding)
      wmat    fp32 [cin, 128]       weight
      iota16  fp32 [128, 16]        constant row 0..15 in every partition
    DRAM output:
      outT    fp32 [128, n_core_pad]   (= (y @ W).T for this core's shard)
    """
    key = (n_rows_pad, cin, n_core_pad, B, C)
    if key in _KERNEL_CACHE:
        return _KERNEL_CACHE[key]

    G = n_core_pad // GRP  # groups per core (multiple of SG)
    S = G // SG  # stripes
    ST = SG * C  # tiles per (stripe, block)
    T = S * B * ST
    KC = cin // 128
    TG = max(1, TGN // cin)  # tiles per gather call (SBUF budget)
    NG = (ST + TG - 1) // TG  # gather calls per (stripe, block)

    nc = bacc.Bacc("TRN2", target_bir_lowering=False, debug=False,
                   num_devices=NCORES,
                   dynamic_dma_scratch_size=int(os.environ.get(
                       "GNN_SCRATCH", "16384")))

    xb = nc.dram_tensor("xb", [B * BLOCK, cin], FP, kind="ExternalInput").ap()
    idxw = nc.dram_tensor("idxw", [16, T, 8], mybir.dt.int16,
                          kind="ExternalInput").ap()
    dstloc = nc.dram_tensor("dstloc", [128, T], FP, kind="ExternalInput").ap()
    nrm = nc.dram_tensor("nrm", [128, T], FP, kind="ExternalInput").ap()
    wmat = nc.dram_tensor("wmat", [cin, 128], FP, kind="ExternalInput").ap()
    iota16 = nc.dram_tensor("iota16", [128, 16], FP, kind="ExternalInput").ap()
    outT = nc.dram_tensor("outT", [128, n_core_pad], FP,
                          kind="ExternalOutput").ap()

    with tile.TileContext(nc) as tc:
        with (
            tc.tile_pool(name="const", bufs=1) as constp,
            tc.tile_pool(name="meta", bufs=2) as metap,
            tc.tile_pool(name="gath", bufs=3) as gathp,
            tc.tile_pool(name="nh", bufs=4) as nhp,
            tc.tile_pool(name="ps", bufs=3, space="PSUM") as psp,
            tc.tile_pool(name="y", bufs=1) as yp,
            tc.tile_pool(name="fin", bufs=2, space="PSUM") as finp,
            tc.tile_pool(name="ob", bufs=2) as obp,
        ):
            rgath = nc.gpsimd.to_reg(TG * 128)
            iota_t = constp.tile([128, 16], FP)
            nc.sync.dma_start(iota_t[:], iota16[:])
            w_t = []
            for kc in range(KC):
                wt = constp.tile([128, 128], FP, name=f"wt{kc}")
                nc.sync.dma_start(wt[:], wmat[kc * 128:(kc + 1) * 128, :])
                w_t.append(wt)
            y_t = [yp.tile([128, n_core_pad], FP, name=f"yt{kc}")
                   for kc in range(KC)]
            for kc in range(KC):
                nc.gpsimd.memset(y_t[kc][:], 0.0)

            # tile t (DRAM metadata order) = ((s*B + b)*SG + gi)*C + c
            for s in range(S):
                st0 = s * B * ST
                idx_sb = metap.tile([128, B * ST, 8], mybir.dt.int16)
                for k in range(8):
                    nc.sync.dma_start(idx_sb[16 * k:16 * (k + 1), :, :],
                                      idxw[:, st0:st0 + B * ST, :])
                dl_sb = metap.tile([128, B * ST], FP)
                nc.sync.dma_start(dl_sb[:], dstloc[:, st0:st0 + B * ST])
                nm_sb = metap.tile([128, B * ST], FP)
                nc.sync.dma_start(nm_sb[:], nrm[:, st0:st0 + B * ST])

                for b in range(B):
                    for ng in range(NG):
                        j0 = b * ST + ng * TG  # within-stripe tile idx
                        ntile = min(TG, ST - ng * TG)
                        gt = gathp.tile([128, TG, cin], FP)
                        nc.gpsimd.dma_gather(
                            gt[:, 0:ntile, :],
                            xb[b * BLOCK:(b + 1) * BLOCK, :],
                            idx_sb[:, j0:j0 + ntile, :],
                            ntile * 128, rgath, cin)
                        for jt in range(ntile):
                            j = j0 + jt
                            q = j % ST  # = gi*C + c
                            gi = q // C
                            goff = (s * SG + gi) * 16
                            nh_t = nhp.tile([128, 16], FP)
                            nc.vector.tensor_scalar(
                                nh_t[:], iota_t[:], dl_sb[:, j:j + 1],
                                nm_sb[:, j:j + 1],
                                mybir.AluOpType.is_equal,
                                mybir.AluOpType.mult)
                            for kc in range(KC):
                                ps_t = psp.tile([128, 16], mybir.dt.float32,
                                                name=f"ps{kc}")
                                nc.tensor.matmul(
                                    ps_t[:],
                                    gt[:, jt, kc * 128:(kc + 1) * 128],
                                    nh_t[:], start=True, stop=True)
                                nc.vector.tensor_add(
                                    y_t[kc][:, goff:goff + 16],
                                    y_t[kc][:, goff:goff + 16], ps_t[:])

            # final: outT = W.T @ yT  (accumulate over channel blocks)
            FC = 512
            q0 = 0
            while q0 < n_core_pad:
                fq = min(FC, n_core_pad - q0)
                fp_t = finp.tile([128, fq], mybir.dt.float32)
                for kc in range(KC):
                    nc.tensor.matmul(fp_t[:], w_t[kc][:],
                                     y_t[kc][:, q0:q0 + fq],
                                     start=(kc == 0), stop=(kc == KC - 1))
                ob_t = obp.tile([128, fq], FP)
                nc.scalar.copy(ob_t[:], fp_t[:])
                nc.sync.dma_start(outT[:, q0:q0 + fq], ob_t[:])
                q0 += fq

    nc.compile()
    _KERNEL_CACHE[key] = nc
    return nc


# ---------------------------------------------------------------------------
# Host-side metadata build for one graph level
# ---------------------------------------------------------------------------
def build_level_meta(src, dst, n):
    """src/dst: active edges (compacted) int arrays; n nodes at this level.
    Returns dict with per-core metadata + shapes + norm info."""
    n_core_pad = int(math.ceil(n / NCORES / (GRP * SG))) * GRP * SG
    n_pad = n_core_pad * NCORES
    B = int(math.ceil(n / BLOCK))
    n_rows_pad = B * BLOCK

    deg = (np.bincount(dst, minlength=n) + 1.0).astype(NPFP)
    dinv = (1.0 / np.sqrt(deg)).astype(NPFP)
    enorm = (dinv[src] * dinv[dst]).astype(NPFP)

    a_src = np.concatenate([src, np.arange(n, dtype=np.int64)])
    a_dst = np.concatenate([dst, np.arange(n, dtype=np.int64)])
    a_nrm = np.concatenate([enorm, (1.0 / deg).astype(NPFP)])

    core = a_dst // n_core_pad
    grp = (a_dst % n_core_pad) // GRP
    blk = a_src // BLOCK
    # cell id global: ((core*G + grp)*B + blk)
    G = n_core_pad // GRP
    cell = (core * G + grp) * B + blk
    order = np.argsort(cell, kind="stable")
    cell_s = cell[order]
    counts = np.bincount(cell_s, minlength=NCORES * G * B)
    C = int(math.ceil(counts.max() / 128.0))
    cap = C * 128
    # position within cell
    cum = np.concatenate([[0], np.cumsum(counts)])[:-1]
    pos = np.arange(len(cell_s)) - cum[cell_s]
    # device tile order: t = ((s*B + b)*SG + gi)*C + c  (stripe-major)
    core_s = cell_s // (G * B)
    grp_s = (cell_s // B) % G
    blk_s = cell_s % B
    st_s = grp_s // SG
    gi_s = grp_s % SG
    T = (G // SG) * B * SG * C  # per core
    tbase = ((st_s * B + blk_s) * SG + gi_s) * C
    slot = (core_s * T + tbase) * 128 + pos
    tot = NCORES * T * 128
    idx_all = np.zeros(tot, dtype=np.int16)
    dl_all = np.zeros(tot, dtype=NPFP)
    nm_all = np.zeros(tot, dtype=NPFP)
    idx_all[slot] = (a_src[order] % BLOCK).astype(np.int16)
    dl_all[slot] = (a_dst[order] % GRP).astype(NPFP)
    nm_all[slot] = a_nrm[order]

    idx_all = idx_all.reshape(NCORES, T, 128)
    dl_all = dl_all.reshape(NCORES, T, 128)
    nm_all = nm_all.reshape(NCORES, T, 128)

    per_core = []
    for c in range(NCORES):
        # wrapped idx layout: [16, T, 8]; idx i of tile t at [i%16, t, i//16]
        idxw = idx_all[c].reshape(T, 8, 16).transpose(2, 0, 1).copy()
        dl = dl_all[c].transpose(1, 0).copy()  # [128, T]
        nm = nm_all[c].transpose(1, 0).copy()
        per_core.append({"idxw": idxw, "dstloc": dl, "nrm": nm})
    return {
        "per_core": per_core, "n": n, "n_pad": n_pad,
        "n_core_pad": n_core_pad, "n_rows_pad": n_rows_pad,
        "B": B, "C": C, "T": T, "deg": deg,
    }


IOTA16 = np.broadcast_to(np.arange(16, dtype=NPFP), (128, 16)).copy()

EXEC_NS = []  # accumulated HW exec times when tracing enabled


def run_conv(meta, x_full, W):
    """x_full: [n, cin] fp32 (full, unpadded); W: [cin, 128].
    Returns y_out [n, 128] fp32 = GCN aggregation @ W (no bias)."""
    cin = x_full.shape[1]
    nc = build_conv_kernel(meta["n_rows_pad"], cin, meta["n_core_pad"],
                           meta["B"], meta["C"])
    xb = np.zeros((meta["n_rows_pad"], cin), dtype=NPFP)
    xb[:meta["n"]] = x_full
    Wf = np.ascontiguousarray(W.astype(NPFP))
    in_maps = []
    for c in range(NCORES):
        pc = meta["per_core"][c]
        in_maps.append({
            "xb": xb, "idxw": pc["idxw"], "dstloc": pc["dstloc"],
            "nrm": pc["nrm"], "wmat": Wf, "iota16": IOTA16,
        })
    trace = bool(int(os.environ.get("GNN_TRACE", "0")))
    res = bass_utils.run_bass_kernel_spmd(
        nc, in_maps, core_ids=list(range(NCORES)), trace=trace)
    if res.exec_time_ns is not None:
        EXEC_NS.append(res.exec_time_ns)
    outs = [r["outT"] for r in res.results]
    y = np.concatenate([o.T for o in outs], axis=0)  # [n_pad, 128]
    return y[:meta["n"]]


# ---------------------------------------------------------------------------
# Host reference pieces (numpy, matching reference.py semantics)
# ---------------------------------------------------------------------------
def bn_relu(x, g, beta):
    m = x.mean(axis=0, dtype=np.float64).astype(NPFP)
    v = ((x - m) ** 2).mean(axis=0, dtype=np.float64).astype(NPFP)
    out = (x - m) * (1.0 / np.sqrt(v + EPS)) * g + beta
    return np.maximum(out, 0.0).astype(NPFP)


def topk_host(score, k):
    # match jax.lax.top_k: descending values, ties -> lower index first
    idx = np.argsort(-score, kind="stable")[:k]
    return idx.astype(np.int64)


def kernel(x, edge_index, in_W, in_b, dn_W, dn_b, dn_g, dn_beta, pool_w,
           bot_W, bot_b, up_W, up_b, up_g, up_beta, out_W, out_b):
    x = np.asarray(x, dtype=NPFP)
    src = np.asarray(edge_index[0], dtype=np.int64)
    dst = np.asarray(edge_index[1], dtype=np.int64)
    n = x.shape[0]

    meta0 = build_level_meta(src, dst, n)
    metas = [meta0]

    # in conv
    x = run_conv(meta0, x, np.asarray(in_W)) + np.asarray(in_b, dtype=NPFP)

    xs, stack = [], []
    cur_src, cur_dst, cur_n, cur_meta = src, dst, n, meta0
    for i in range(DEPTH):
        x = run_conv(cur_meta, x, np.asarray(dn_W[i])) + \
            np.asarray(dn_b[i], dtype=NPFP)
        x = bn_relu(x, np.asarray(dn_g[i], dtype=NPFP),
                    np.asarray(dn_beta[i], dtype=NPFP))
        xs.append(x)
        k = int(RATIO * cur_n)
        w = np.asarray(pool_w[i], dtype=NPFP)
        score = np.tanh(x @ w / np.sqrt((w * w).sum()))
        idx = topk_host(score, k)
        new_id = np.zeros(cur_n, dtype=np.int64)
        new_id[idx] = np.arange(k)
        kept = np.zeros(cur_n, dtype=bool)
        kept[idx] = True
        emask = kept[cur_src] & kept[cur_dst]
        stack.append((cur_meta, idx, cur_n))
        cur_src = new_id[cur_src[emask]]
        cur_dst = new_id[cur_dst[emask]]
        cur_n = k
        x = x[idx]
        cur_meta = build_level_meta(cur_src, cur_dst, cur_n)
        metas.append(cur_meta)

    # bottleneck
    x = run_conv(cur_meta, x, np.asarray(bot_W)) + \
        np.asarray(bot_b, dtype=NPFP)
    x = np.maximum(x, 0.0)

    for i in range(DEPTH):
        p_meta, idx, pn = stack[DEPTH - 1 - i]
        xf = np.zeros((pn, x.shape[1]), dtype=NPFP)
        xf[idx] = x
        x = np.concatenate([xf, xs[DEPTH - 1 - i]], axis=1)
        x = run_conv(p_meta, x, np.asarray(up_W[i])) + \
            np.asarray(up_b[i], dtype=NPFP)
        x = bn_relu(x, np.asarray(up_g[i], dtype=NPFP),
                    np.asarray(up_beta[i], dtype=NPFP))
        cur_meta = p_meta

    out = run_conv(cur_meta, x, np.asarray(out_W)) + \
        np.asarray(out_b, dtype=NPFP)
    return out.astype(np.float32)



# revision 4
# speedup vs baseline: 2.6385x; 2.6385x over previous
"""GraphUNet Trainium kernel (v2).

Architecture: 9 GCN convs (in, dn0, dn1, dn2, bottom, up0, up1, up2, out)
with top-k pooling / unpooling and batch-norm+relu between convs.

Device per conv: edge aggregation y[d] = sum_{e: dst=d} norm_e * x[src_e]
 + self-loop x[d]/deg[d], then out = (y @ W).T via:
  - dma_gather of x rows (fp32, 4 SWDGE queues round-robin) in tiles of
    128 edges packed by a joint-greedy sliding-window schedule (shared
    across the 8 SPMD cores; per-core slot data fills the tiles),
  - fp16 one-hot matmul: psum[128c x 64] += x_rows^T @ onehot(dst)*norm,
  - self-loop term via sequential shard stream + diagonal matmul
    (no gather descriptors; also serves as the y initializer),
  - final fp32 matmul streaming y through W.

Host (numpy): sharding/metadata build, top-k pools, edge relabeling,
degree/norm precompute, batch-norm, relu, bias, concat.

Sharding: dst-node ranges across 8 cores; x replicated to all cores
(graph/data parallel; halo exchange realized as full replication of the
per-conv feature table, re-staged by the host between launches).
"""

import math
import os
import sys

import numpy as np

sys.path.insert(0, "/opt/trn_rl_repo")

import concourse.bass as bass  # noqa: E402,F401
import concourse.bacc as bacc  # noqa: E402
import concourse.tile as tile  # noqa: E402
from concourse import mybir  # noqa: E402
from concourse import bass_utils  # noqa: E402

# ---- problem constants (hardcoded per task statement) ----
C_IN = 128
H = 128
DEPTH = 3
RATIO = 0.5
EPS = 1e-5
NCORES = 8
W = 64            # one-hot dst window width
TG = 8            # tiles per gather call (1024 idxs = SWDGE ring limit)
SM = 768          # tiles per metadata stripe (multiple of TG)
NQ = 4            # SWDGE queues (desc-gen parallelism)
IDXMAX = 32768    # int16 gather index reach

FP = mybir.dt.float32
FH = mybir.dt.float16
NPF = np.float32
NPH = np.float16

IOTA = np.broadcast_to(np.arange(128, dtype=NPH), (128, 128)).copy()
PIDX = np.arange(128, dtype=NPF).reshape(128, 1).copy()

EXEC_NS = []  # accumulated HW exec times when tracing enabled


# ---------------------------------------------------------------------------
# Bass kernel builder (one conv shape + baked tile schedule).
# ---------------------------------------------------------------------------
_KERNEL_CACHE = {}


def build_conv_kernel(cin, B, BLOCK, ncp, sched, groups):
    """sched: list of (block, goff) per tile; groups: list of (t0, nt, b).

    DRAM inputs (per core):
      xb    fp32 [B*BLOCK, cin]  full (padded) node features
      xs    fp32 [ncp, cin]      this core's dst-shard rows (padded)
      idxw  int16 [16, T, 8]     wrapped per-tile local src indices
      dl    fp32 [128, T]        per-slot dst offset within window (0..W-1)
      nm    fp32 [128, T]        per-slot edge norm (0 for padding)
      vdeg  fp32 [128, NC128]    1/deg for shard nodes (wrapped by chunk)
      wmat  fp32 [cin, 128]      weight
      iota  fp16 [128, 128]      col j = j in every partition
      pidx  fp32 [128, 1]        partition index column
    DRAM output:
      outT  fp32 [128, ncp]      (= (y @ W).T for this core's shard)
    """
    T = len(sched)
    key = (cin, B, BLOCK, ncp, hash(tuple(sched)))
    if key in _KERNEL_CACHE:
        return _KERNEL_CACHE[key]

    KC = cin // 128
    NC128 = ncp // 128

    nc = bacc.Bacc("TRN2", target_bir_lowering=False, debug=False,
                   num_devices=NCORES, num_swdge_queues=NQ,
                   dynamic_dma_scratch_size=16384)

    xb = nc.dram_tensor("xb", [B * BLOCK, cin], FP, kind="ExternalInput").ap()
    xs = nc.dram_tensor("xs", [ncp, cin], FP, kind="ExternalInput").ap()
    Tm = max(T, 1)  # zero-size DRAM tensors are awkward; keep >=1
    idxw = nc.dram_tensor("idxw", [16, Tm, 8], mybir.dt.int16,
                          kind="ExternalInput").ap()
    dl = nc.dram_tensor("dl", [128, Tm], FP, kind="ExternalInput").ap()
    nm = nc.dram_tensor("nm", [128, Tm], FP, kind="ExternalInput").ap()
    vdeg = nc.dram_tensor("vdeg", [128, NC128], FP, kind="ExternalInput").ap()
    wmat = nc.dram_tensor("wmat", [cin, 128], FP, kind="ExternalInput").ap()
    iota = nc.dram_tensor("iota", [128, 128], FH, kind="ExternalInput").ap()
    pidx = nc.dram_tensor("pidx", [128, 1], FP, kind="ExternalInput").ap()
    outT = nc.dram_tensor("outT", [128, ncp], FP, kind="ExternalOutput").ap()

    with tile.TileContext(nc) as tc:
        with (
            tc.tile_pool(name="const", bufs=1) as constp,
            tc.tile_pool(name="y", bufs=1) as yp,
            tc.tile_pool(name="meta", bufs=2) as metap,
            tc.tile_pool(name="xsl", bufs=3) as xslp,
            tc.tile_pool(name="gath", bufs=3) as gathp,
            tc.tile_pool(name="ghalf", bufs=3) as ghp,
            tc.tile_pool(name="nh", bufs=6) as nhp,
            tc.tile_pool(name="ps", bufs=4, space="PSUM") as psp,
            tc.tile_pool(name="sf", bufs=2, space="PSUM") as sfp,
            tc.tile_pool(name="fin", bufs=2, space="PSUM") as finp,
            tc.tile_pool(name="ob", bufs=2) as obp,
        ):
            iota_t = constp.tile([128, 128], FH)
            nc.sync.dma_start(iota_t[:], iota[:])
            pidx_t = constp.tile([128, 1], FP)
            nc.sync.dma_start(pidx_t[:], pidx[:])
            vdeg_t = constp.tile([128, NC128], FP)
            nc.sync.dma_start(vdeg_t[:], vdeg[:])
            w_t = []
            for kc in range(KC):
                wt = constp.tile([128, 128], FP, name=f"wt{kc}")
                nc.sync.dma_start(wt[:], wmat[kc * 128:(kc + 1) * 128, :])
                w_t.append(wt)
            y_t = [yp.tile([128, ncp], FP, name=f"yt{kc}") for kc in range(KC)]

            regs = {}
            for nt in sorted({g[1] for g in groups}):
                regs[nt] = nc.gpsimd.to_reg(nt * 128)

            # ---- self-loop pass: y[:, chunk] = (xs_chunk^T) @ diag(1/deg)
            # (initializes y; pad chunks have vdeg=0 so y pad cols are 0)
            for k in range(NC128):
                xst = xslp.tile([128, cin], FP)
                nc.sync.dma_start(xst[:], xs[k * 128:(k + 1) * 128, :])
                xsh = xslp.tile([128, cin], FH)
                nc.scalar.copy(xsh[:], xst[:])
                Dt = nhp.tile([128, 128], FH)
                nc.vector.tensor_scalar(
                    Dt[:], iota_t[:], pidx_t[:, 0:1], vdeg_t[:, k:k + 1],
                    mybir.AluOpType.is_equal, mybir.AluOpType.mult)
                for kc in range(KC):
                    sf_t = sfp.tile([128, 128], mybir.dt.float32)
                    nc.tensor.matmul(sf_t[:], xsh[:, kc * 128:(kc + 1) * 128],
                                     Dt[:], start=True, stop=True)
                    nc.scalar.copy(y_t[kc][:, k * 128:(k + 1) * 128], sf_t[:])

            # ---- edge tiles (gather + one-hot matmul accumulate)
            idx_sb = dl_sb = nm_sb = None
            s0 = -1
            for gi, (t0, nt, b) in enumerate(groups):
                if t0 // SM != s0:
                    s0 = t0 // SM
                    st0 = s0 * SM
                    stn = min(SM, T - st0)
                    idx_sb = metap.tile([128, SM, 8], mybir.dt.int16)
                    for k in range(8):
                        nc.sync.dma_start(idx_sb[16 * k:16 * (k + 1), 0:stn, :],
                                          idxw[:, st0:st0 + stn, :])
                    dl_sb = metap.tile([128, SM], FP)
                    nc.sync.dma_start(dl_sb[:, 0:stn], dl[:, st0:st0 + stn])
                    nm_sb = metap.tile([128, SM], FP)
                    nc.sync.dma_start(nm_sb[:, 0:stn], nm[:, st0:st0 + stn])
                lo = t0 - st0
                gt = gathp.tile([128, TG, cin], FP)
                nc.gpsimd.dma_gather(
                    gt[:, 0:nt, :], xb[b * BLOCK:(b + 1) * BLOCK, :],
                    idx_sb[:, lo:lo + nt, :], nt * 128, regs[nt], cin,
                    queue_num=gi % NQ)
                ght = ghp.tile([128, TG, cin], FH)
                nc.scalar.copy(ght[:, 0:nt, :], gt[:, 0:nt, :])
                for jt in range(nt):
                    tt = t0 + jt
                    goff = sched[tt][1]
                    sl = lo + jt
                    nh_t = nhp.tile([128, W], FH)
                    nc.vector.tensor_scalar(
                        nh_t[:], iota_t[:, 0:W], dl_sb[:, sl:sl + 1],
                        nm_sb[:, sl:sl + 1],
                        mybir.AluOpType.is_equal, mybir.AluOpType.mult)
                    for kc in range(KC):
                        ps_t = psp.tile([128, W], mybir.dt.float32)
                        nc.tensor.matmul(
                            ps_t[:], ght[:, jt, kc * 128:(kc + 1) * 128],
                            nh_t[:], start=True, stop=True)
                        nc.vector.tensor_add(
                            y_t[kc][:, goff:goff + W],
                            y_t[kc][:, goff:goff + W], ps_t[:])

            # ---- final: outT = sum_kc W[kc].T @ y[kc]
            FC = 512
            q0 = 0
            while q0 < ncp:
                fq = min(FC, ncp - q0)
                fp_t = finp.tile([128, fq], mybir.dt.float32)
                for kc in range(KC):
                    nc.tensor.matmul(fp_t[:], w_t[kc][:],
                                     y_t[kc][:, q0:q0 + fq],
                                     start=(kc == 0), stop=(kc == KC - 1))
                ob_t = obp.tile([128, fq], FP)
                nc.scalar.copy(ob_t[:], fp_t[:])
                nc.sync.dma_start(outT[:, q0:q0 + fq], ob_t[:])
                q0 += fq

    nc.compile()
    _KERNEL_CACHE[key] = nc
    return nc


# ---------------------------------------------------------------------------
# Host-side metadata build for one graph level
# ---------------------------------------------------------------------------
def build_level_meta(src, dst, n):
    """src/dst: active edges (compacted, relabeled) int64 arrays; n nodes.

    Builds the SPMD-shared joint-greedy tile schedule + per-core slot data.
    """
    n_core = (n + NCORES - 1) // NCORES
    ncp = max(128, ((n_core + 127) // 128) * 128)
    B = max(1, (n + IDXMAX - 1) // IDXMAX)
    BLOCK = ((n + B - 1) // B + 7) // 8 * 8
    assert BLOCK <= IDXMAX

    deg = (np.bincount(dst, minlength=n) + 1.0).astype(NPF)
    dinv = (1.0 / np.sqrt(deg)).astype(NPF)
    enorm = (dinv[src] * dinv[dst]).astype(NPF)

    core = dst // n_core
    dloc = (dst - core * n_core).astype(np.int64)
    blk = src // BLOCK

    order = np.lexsort((dloc, blk, core))
    sc = src[order]
    dc = dloc[order]
    wc = enorm[order]
    keys = (core[order] * B + blk[order]).astype(np.int64)

    # segment bounds per (core, block)
    bounds = np.searchsorted(keys, np.arange(NCORES * B + 1))

    sched = []          # (block, goff)
    tile_take = []      # per tile: list of (core, pos, cnt)
    for b in range(B):
        pos = [int(bounds[c * B + b]) for c in range(NCORES)]
        hi = [int(bounds[c * B + b + 1]) for c in range(NCORES)]
        while True:
            base = None
            for c in range(NCORES):
                if pos[c] < hi[c]:
                    v = int(dc[pos[c]])
                    if base is None or v < base:
                        base = v
            if base is None:
                break
            base = min(base, ncp - W)
            takes = []
            for c in range(NCORES):
                if pos[c] < hi[c]:
                    e = pos[c] + int(np.searchsorted(
                        dc[pos[c]:hi[c]], base + W, side="left"))
                    cnt = min(128, e - pos[c])
                    if cnt > 0:
                        takes.append((c, pos[c], cnt))
                        pos[c] += cnt
            sched.append((b, base))
            tile_take.append(takes)

    T = len(sched)
    Tm = max(T, 1)
    idx16 = np.zeros((NCORES, Tm, 128), dtype=np.int16)
    dl_a = np.zeros((NCORES, Tm, 128), dtype=NPF)
    nm_a = np.zeros((NCORES, Tm, 128), dtype=NPF)
    for t, takes in enumerate(tile_take):
        bb, goff = sched[t]
        for c, p0, cnt in takes:
            sl = slice(p0, p0 + cnt)
            idx16[c, t, :cnt] = (sc[sl] - bb * BLOCK).astype(np.int16)
            dl_a[c, t, :cnt] = (dc[sl] - goff).astype(NPF)
            nm_a[c, t, :cnt] = wc[sl].astype(NPF)

    # groups: runs of <=TG tiles, same block, within one stripe
    groups = []
    t = 0
    while t < T:
        b = sched[t][0]
        bend = t
        while bend < T and sched[bend][0] == b:
            bend += 1
        while t < bend:
            nt = min(TG, bend - t, (t // SM + 1) * SM - t)
            groups.append((t, nt, b))
            t += nt

    NC128 = ncp // 128
    per_core = []
    for c in range(NCORES):
        idxw = idx16[c].reshape(Tm, 8, 16).transpose(2, 0, 1).copy()
        dlw = dl_a[c].transpose(1, 0).copy()
        nmw = nm_a[c].transpose(1, 0).copy()
        vd = np.zeros((128, NC128), dtype=NPF)
        lo = c * n_core
        cnt = max(0, min(n - lo, n_core))
        if cnt > 0:
            col = np.zeros(NC128 * 128, dtype=NPF)
            col[:cnt] = 1.0 / deg[lo:lo + cnt]
            vd[:, :] = col.reshape(NC128, 128).T
        per_core.append({"idxw": idxw, "dl": dlw, "nm": nmw, "vdeg": vd})

    return {
        "per_core": per_core, "n": n, "n_core": n_core, "ncp": ncp,
        "B": B, "BLOCK": BLOCK, "T": T, "sched": tuple(sched),
        "groups": groups, "deg": deg,
    }


def run_conv(meta, x_full, Wmat):
    """x_full: [n, cin] fp32 (full, unpadded); Wmat: [cin, 128].
    Returns y_out [n, 128] fp32 = GCN aggregation @ W (no bias)."""
    cin = x_full.shape[1]
    nc = build_conv_kernel(cin, meta["B"], meta["BLOCK"], meta["ncp"],
                           meta["sched"], meta["groups"])
    n, n_core, ncp = meta["n"], meta["n_core"], meta["ncp"]
    xb = np.zeros((meta["B"] * meta["BLOCK"], cin), dtype=NPF)
    xb[:n] = x_full
    Wf = np.ascontiguousarray(Wmat.astype(NPF))
    in_maps = []
    for c in range(NCORES):
        pc = meta["per_core"][c]
        lo = c * n_core
        xsc = np.zeros((ncp, cin), dtype=NPF)
        cnt = max(0, min(n - lo, n_core))
        if cnt > 0:
            xsc[:cnt] = x_full[lo:lo + cnt]
        in_maps.append({
            "xb": xb, "xs": xsc, "idxw": pc["idxw"], "dl": pc["dl"],
            "nm": pc["nm"], "vdeg": pc["vdeg"], "wmat": Wf,
            "iota": IOTA, "pidx": PIDX,
        })
    trace = bool(int(os.environ.get("GNN_TRACE", "0")))
    res = bass_utils.run_bass_kernel_spmd(
        nc, in_maps, core_ids=list(range(NCORES)), trace=trace)
    if res.exec_time_ns is not None:
        EXEC_NS.append(res.exec_time_ns)
    outs = [r["outT"] for r in res.results]
    y = np.concatenate([o.T for o in outs], axis=0)  # [8*ncp, 128]
    if ncp != n_core:
        y = y.reshape(NCORES, ncp, 128)[:, :n_core].reshape(-1, 128)
    return np.ascontiguousarray(y[:n])


# ---------------------------------------------------------------------------
# Host reference pieces (numpy, matching reference.py semantics)
# ---------------------------------------------------------------------------
def bn_relu(x, g, beta):
    m = x.mean(axis=0, dtype=np.float64).astype(NPF)
    v = ((x - m) ** 2).mean(axis=0, dtype=np.float64).astype(NPF)
    out = (x - m) * (1.0 / np.sqrt(v + EPS)) * g + beta
    return np.maximum(out, 0.0).astype(NPF)


def topk_host(score, k):
    # match jax.lax.top_k: descending values, ties -> lower index first
    idx = np.argsort(-score, kind="stable")[:k]
    return idx.astype(np.int64)


def kernel(x, edge_index, in_W, in_b, dn_W, dn_b, dn_g, dn_beta, pool_w,
           bot_W, bot_b, up_W, up_b, up_g, up_beta, out_W, out_b):
    x = np.asarray(x, dtype=NPF)
    src = np.asarray(edge_index[0], dtype=np.int64)
    dst = np.asarray(edge_index[1], dtype=np.int64)
    n = x.shape[0]

    meta0 = build_level_meta(src, dst, n)

    # in conv
    x = run_conv(meta0, x, np.asarray(in_W)) + np.asarray(in_b, dtype=NPF)

    xs, stack = [], []
    cur_src, cur_dst, cur_n, cur_meta = src, dst, n, meta0
    for i in range(DEPTH):
        x = run_conv(cur_meta, x, np.asarray(dn_W[i])) + \
            np.asarray(dn_b[i], dtype=NPF)
        x = bn_relu(x, np.asarray(dn_g[i], dtype=NPF),
                    np.asarray(dn_beta[i], dtype=NPF))
        xs.append(x)
        k = int(RATIO * cur_n)
        w = np.asarray(pool_w[i], dtype=NPF)
        score = np.tanh(x @ w / np.sqrt((w * w).sum()))
        idx = topk_host(score, k)
        new_id = np.zeros(cur_n, dtype=np.int64)
        new_id[idx] = np.arange(k)
        kept = np.zeros(cur_n, dtype=bool)
        kept[idx] = True
        emask = kept[cur_src] & kept[cur_dst]
        stack.append((cur_meta, idx, cur_n))
        cur_src = new_id[cur_src[emask]]
        cur_dst = new_id[cur_dst[emask]]
        cur_n = k
        x = x[idx]
        cur_meta = build_level_meta(cur_src, cur_dst, cur_n)

    # bottleneck
    x = run_conv(cur_meta, x, np.asarray(bot_W)) + \
        np.asarray(bot_b, dtype=NPF)
    x = np.maximum(x, 0.0)

    for i in range(DEPTH):
        p_meta, idx, pn = stack[DEPTH - 1 - i]
        xf = np.zeros((pn, x.shape[1]), dtype=NPF)
        xf[idx] = x
        x = np.concatenate([xf, xs[DEPTH - 1 - i]], axis=1)
        x = run_conv(p_meta, x, np.asarray(up_W[i])) + \
            np.asarray(up_b[i], dtype=NPF)
        x = bn_relu(x, np.asarray(up_g[i], dtype=NPF),
                    np.asarray(up_beta[i], dtype=NPF))
        cur_meta = p_meta

    out = run_conv(cur_meta, x, np.asarray(out_W)) + \
        np.asarray(out_b, dtype=NPF)
    return out.astype(np.float32)


# revision 11
# speedup vs baseline: 4.6642x; 1.7677x over previous
"""GraphUNet Trainium kernel (v2).

Architecture: 9 GCN convs (in, dn0, dn1, dn2, bottom, up0, up1, up2, out)
with top-k pooling / unpooling and batch-norm+relu between convs.

Device per conv: edge aggregation y[d] = sum_{e: dst=d} norm_e * x[src_e]
 + self-loop x[d]/deg[d], then out = (y @ W).T via:
  - dma_gather of x rows (fp32, 4 SWDGE queues round-robin) in tiles of
    128 edges packed by a joint-greedy sliding-window schedule (shared
    across the 8 SPMD cores; per-core slot data fills the tiles),
  - fp16 one-hot matmul: psum[128c x 64] += x_rows^T @ onehot(dst)*norm,
  - self-loop term via sequential shard stream + diagonal matmul
    (no gather descriptors; also serves as the y initializer),
  - final fp32 matmul streaming y through W.

Host (numpy): sharding/metadata build, top-k pools, edge relabeling,
degree/norm precompute, batch-norm, relu, bias, concat.

Sharding: dst-node ranges across 8 cores; x replicated to all cores
(graph/data parallel; halo exchange realized as full replication of the
per-conv feature table, re-staged by the host between launches).
"""

import math
import os
import sys

import numpy as np

sys.path.insert(0, "/opt/trn_rl_repo")

import concourse.bass as bass  # noqa: E402,F401
import concourse.bacc as bacc  # noqa: E402
import concourse.tile as tile  # noqa: E402
from concourse import mybir  # noqa: E402
from concourse import bass_utils  # noqa: E402

# ---- problem constants (hardcoded per task statement) ----
C_IN = 128
H = 128
DEPTH = 3
RATIO = 0.5
EPS = 1e-5
NCORES = 8
W = 64            # one-hot dst window width
TG = 8            # tiles per gather call (1024 idxs = SWDGE ring limit)
SM = 768          # tiles per metadata stripe (multiple of TG)
NQ = 4            # SWDGE queues (desc-gen parallelism)
IDXMAX = 32768    # int16 gather index reach

FP = mybir.dt.float32
FH = mybir.dt.float16
NPF = np.float32
NPH = np.float16

IOTA = np.broadcast_to(np.arange(128, dtype=NPH), (128, 128)).copy()
PIDX = np.arange(128, dtype=NPF).reshape(128, 1).copy()
IOTA3 = np.broadcast_to(np.arange(W, dtype=NPF), (128, TG, W)).reshape(
    128, TG * W).copy()
ZROS = np.zeros((128, 512), dtype=NPH)
GSPAN = 512       # group psum window width (one PSUM bank)

EXEC_NS = []  # accumulated HW exec times when tracing enabled


# ---------------------------------------------------------------------------
# Bass kernel builder (one conv shape + baked tile schedule).
# ---------------------------------------------------------------------------
_KERNEL_CACHE = {}


def build_conv_kernel(cin, B, BLOCK, ncp, sched, groups):
    """sched: list of (block, goff) per tile; groups: list of (t0, nt, b).

    DRAM inputs (per core):
      xb    fp32 [B*BLOCK, cin]  full (padded) node features
      xs    fp32 [ncp, cin]      this core's dst-shard rows (padded)
      idxw  int16 [16, T, 8]     wrapped per-tile local src indices
      dl    fp32 [128, T]        per-slot dst offset within window (0..W-1)
      nm    fp32 [128, T]        per-slot edge norm (0 for padding)
      vdeg  fp32 [128, NC128]    1/deg for shard nodes (wrapped by chunk)
      wmat  fp32 [cin, 128]      weight
      iota  fp16 [128, 128]      col j = j in every partition
      pidx  fp32 [128, 1]        partition index column
    DRAM output:
      outT  fp32 [128, ncp]      (= (y @ W).T for this core's shard)
    """
    T = len(sched)
    key = (cin, B, BLOCK, ncp, hash(tuple(sched)))
    if key in _KERNEL_CACHE:
        return _KERNEL_CACHE[key]

    KC = cin // 128
    NC128 = ncp // 128

    nc = bacc.Bacc("TRN2", target_bir_lowering=False, debug=False,
                   num_devices=NCORES, num_swdge_queues=NQ,
                   dynamic_dma_scratch_size=16384)

    xb = nc.dram_tensor("xb", [B * BLOCK, cin], FP, kind="ExternalInput").ap()
    xs = nc.dram_tensor("xs", [ncp, cin], FP, kind="ExternalInput").ap()
    Tm = max(T, 1)  # zero-size DRAM tensors are awkward; keep >=1
    idxw = nc.dram_tensor("idxw", [16, Tm, 8], mybir.dt.int16,
                          kind="ExternalInput").ap()
    dl = nc.dram_tensor("dl", [128, Tm], FP, kind="ExternalInput").ap()
    nm = nc.dram_tensor("nm", [128, Tm], FP, kind="ExternalInput").ap()
    vdeg = nc.dram_tensor("vdeg", [128, NC128], FP, kind="ExternalInput").ap()
    wmat = nc.dram_tensor("wmat", [cin, 128], FP, kind="ExternalInput").ap()
    iota = nc.dram_tensor("iota", [128, 128], FH, kind="ExternalInput").ap()
    iota3 = nc.dram_tensor("iota3", [128, TG * W], FP,
                           kind="ExternalInput").ap()
    zro = nc.dram_tensor("zro", [128, GSPAN], FH, kind="ExternalInput").ap()
    pidx = nc.dram_tensor("pidx", [128, 1], FP, kind="ExternalInput").ap()
    outT = nc.dram_tensor("outT", [128, ncp], FP, kind="ExternalOutput").ap()

    with tile.TileContext(nc) as tc:
        with (
            tc.tile_pool(name="const", bufs=1) as constp,
            tc.tile_pool(name="y", bufs=1) as yp,
            tc.tile_pool(name="meta", bufs=2) as metap,
            tc.tile_pool(name="xsl", bufs=3) as xslp,
            tc.tile_pool(name="gath", bufs=5 if cin == 128 else 3) as gathp,
            tc.tile_pool(name="ghalf", bufs=3) as ghp,
            tc.tile_pool(name="nh", bufs=4) as nhp,
            tc.tile_pool(name="eq", bufs=3) as eqp,
            tc.tile_pool(name="pg", bufs=4 if cin == 128 else 2,
                         space="PSUM") as pgp,
            tc.tile_pool(name="sf", bufs=2, space="PSUM") as sfp,
            tc.tile_pool(name="fin", bufs=2, space="PSUM") as finp,
            tc.tile_pool(name="ob", bufs=2) as obp,
        ):
            iota_t = constp.tile([128, 128], FH)
            nc.sync.dma_start(iota_t[:], iota[:])
            iota3_t = constp.tile([128, TG, W], FP)
            nc.sync.dma_start(iota3_t[:], iota3[:])
            zro_t = constp.tile([128, GSPAN], FH)
            nc.sync.dma_start(zro_t[:], zro[:])
            pidx_t = constp.tile([128, 1], FP)
            nc.sync.dma_start(pidx_t[:], pidx[:])
            vdeg_t = constp.tile([128, NC128], FP)
            nc.sync.dma_start(vdeg_t[:], vdeg[:])
            w_t = []
            for kc in range(KC):
                wt = constp.tile([128, 128], FP, name=f"wt{kc}")
                nc.sync.dma_start(wt[:], wmat[kc * 128:(kc + 1) * 128, :])
                w_t.append(wt)
            y_t = [yp.tile([128, ncp], FP, name=f"yt{kc}") for kc in range(KC)]

            regs = {}
            for nt in sorted({g[1] for g in groups}):
                regs[nt] = nc.gpsimd.to_reg(nt * 128)

            # ---- self-loop pass: y[:, chunk] = (xs_chunk^T) @ diag(1/deg)
            # (initializes y; pad chunks have vdeg=0 so y pad cols are 0)
            for k in range(NC128):
                xst = xslp.tile([128, cin], FP)
                nc.sync.dma_start(xst[:], xs[k * 128:(k + 1) * 128, :])
                xsh = xslp.tile([128, cin], FH)
                nc.scalar.copy(xsh[:], xst[:])
                Dt = nhp.tile([128, 128], FH)
                nc.vector.tensor_scalar(
                    Dt[:], iota_t[:], pidx_t[:, 0:1], vdeg_t[:, k:k + 1],
                    mybir.AluOpType.is_equal, mybir.AluOpType.mult)
                for kc in range(KC):
                    sf_t = sfp.tile([128, 128], mybir.dt.float32)
                    nc.tensor.matmul(sf_t[:], xsh[:, kc * 128:(kc + 1) * 128],
                                     Dt[:], start=True, stop=True)
                    nc.scalar.copy(y_t[kc][:, k * 128:(k + 1) * 128], sf_t[:])

            # ---- edge tiles (gather + batched one-hot + group-psum accum)
            nstripe = (T + SM - 1) // SM if T > 0 else 0
            stripe_tiles = {}

            def load_stripe(s):
                st0 = s * SM
                stn = min(SM, T - st0)
                i_sb = metap.tile([128, SM, 8], mybir.dt.int16)
                for k in range(8):
                    nc.sync.dma_start(i_sb[16 * k:16 * (k + 1), 0:stn, :],
                                      idxw[:, st0:st0 + stn, :])
                d_sb = metap.tile([128, SM], FP)
                nc.sync.dma_start(d_sb[:, 0:stn], dl[:, st0:st0 + stn])
                n_sb = metap.tile([128, SM], FP)
                nc.sync.dma_start(n_sb[:, 0:stn], nm[:, st0:st0 + stn])
                stripe_tiles[s] = (i_sb, d_sb, n_sb)

            if nstripe > 0:
                load_stripe(0)
            for gi, (t0, nt, b) in enumerate(groups):
                s0 = t0 // SM
                if s0 + 1 < nstripe and s0 + 1 not in stripe_tiles \
                        and t0 - s0 * SM >= SM // 2:
                    load_stripe(s0 + 1)
                idx_sb, dl_sb, nm_sb = stripe_tiles[s0]
                lo = t0 - s0 * SM
                gbase = sched[t0][1]
                gw = sched[t0 + nt - 1][1] + W - gbase
                gt = gathp.tile([128, TG, cin], FP)
                nc.gpsimd.dma_gather(
                    gt[:, 0:nt, :], xb[b * BLOCK:(b + 1) * BLOCK, :],
                    idx_sb[:, lo:lo + nt, :], nt * 128, regs[nt], cin,
                    queue_num=gi % NQ)
                ght = ghp.tile([128, TG, cin], FH)
                nc.scalar.copy(ght[:, 0:nt, :], gt[:, 0:nt, :])
                eq_t = eqp.tile([128, TG, W], FP)
                dlb = dl_sb[:, lo:lo + nt].unsqueeze(2).broadcast_to(
                    (128, nt, W))
                nmb = nm_sb[:, lo:lo + nt].unsqueeze(2).broadcast_to(
                    (128, nt, W))
                nc.vector.tensor_tensor(eq_t[:, 0:nt, :], iota3_t[:, 0:nt, :],
                                        dlb, mybir.AluOpType.is_equal)
                nh_g = nhp.tile([128, TG, W], FH)
                nc.vector.tensor_tensor(nh_g[:, 0:nt, :], eq_t[:, 0:nt, :],
                                        nmb, mybir.AluOpType.mult)
                pg_t = [pgp.tile([128, GSPAN], mybir.dt.float32,
                                 name=f"pg{kc}") for kc in range(KC)]
                for kc in range(KC):
                    nc.tensor.matmul(pg_t[kc][:, 0:gw], iota_t[:],
                                     zro_t[:, 0:gw], start=True, stop=False,
                                     skip_group_check=True)
                for jt in range(nt):
                    po = sched[t0 + jt][1] - gbase
                    for kc in range(KC):
                        nc.tensor.matmul(
                            pg_t[kc][:, po:po + W],
                            ght[:, jt, kc * 128:(kc + 1) * 128],
                            nh_g[:, jt, :], start=False, stop=(jt == nt - 1),
                            skip_group_check=True)
                for kc in range(KC):
                    nc.vector.tensor_add(
                        y_t[kc][:, gbase:gbase + gw],
                        y_t[kc][:, gbase:gbase + gw], pg_t[kc][:, 0:gw])

            # ---- final: outT = sum_kc W[kc].T @ y[kc]
            FC = 512
            q0 = 0
            while q0 < ncp:
                fq = min(FC, ncp - q0)
                fp_t = finp.tile([128, fq], mybir.dt.float32)
                for kc in range(KC):
                    nc.tensor.matmul(fp_t[:], w_t[kc][:],
                                     y_t[kc][:, q0:q0 + fq],
                                     start=(kc == 0), stop=(kc == KC - 1))
                ob_t = obp.tile([128, fq], FP)
                nc.scalar.copy(ob_t[:], fp_t[:])
                nc.sync.dma_start(outT[:, q0:q0 + fq], ob_t[:])
                q0 += fq

    nc.compile()
    _KERNEL_CACHE[key] = nc
    return nc


# ---------------------------------------------------------------------------
# Host-side metadata build for one graph level
# ---------------------------------------------------------------------------
def build_level_meta(src, dst, n):
    """src/dst: active edges (compacted, relabeled) int64 arrays; n nodes.

    Builds the SPMD-shared joint-greedy tile schedule + per-core slot data.
    """
    n_core = (n + NCORES - 1) // NCORES
    ncp = max(128, ((n_core + 127) // 128) * 128)
    B = max(1, (n + IDXMAX - 1) // IDXMAX)
    BLOCK = ((n + B - 1) // B + 7) // 8 * 8
    assert BLOCK <= IDXMAX

    deg = (np.bincount(dst, minlength=n) + 1.0).astype(NPF)
    dinv = (1.0 / np.sqrt(deg)).astype(NPF)
    enorm = (dinv[src] * dinv[dst]).astype(NPF)

    core = dst // n_core
    dloc = (dst - core * n_core).astype(np.int64)
    blk = src // BLOCK

    order = np.lexsort((dloc, blk, core))
    sc = src[order]
    dc = dloc[order]
    wc = enorm[order]
    keys = (core[order] * B + blk[order]).astype(np.int64)

    # segment bounds per (core, block)
    bounds = np.searchsorted(keys, np.arange(NCORES * B + 1))

    sched = []          # (block, goff)
    tile_take = []      # per tile: list of (core, pos, cnt)
    for b in range(B):
        pos = [int(bounds[c * B + b]) for c in range(NCORES)]
        hi = [int(bounds[c * B + b + 1]) for c in range(NCORES)]
        while True:
            base = None
            for c in range(NCORES):
                if pos[c] < hi[c]:
                    v = int(dc[pos[c]])
                    if base is None or v < base:
                        base = v
            if base is None:
                break
            base = min(base, ncp - W)
            takes = []
            for c in range(NCORES):
                if pos[c] < hi[c]:
                    e = pos[c] + int(np.searchsorted(
                        dc[pos[c]:hi[c]], base + W, side="left"))
                    cnt = min(128, e - pos[c])
                    if cnt > 0:
                        takes.append((c, pos[c], cnt))
                        pos[c] += cnt
            sched.append((b, base))
            tile_take.append(takes)

    T = len(sched)
    Tm = max(T, 1)
    idx16 = np.zeros((NCORES, Tm, 128), dtype=np.int16)
    dl_a = np.zeros((NCORES, Tm, 128), dtype=NPF)
    nm_a = np.zeros((NCORES, Tm, 128), dtype=NPF)
    for t, takes in enumerate(tile_take):
        bb, goff = sched[t]
        for c, p0, cnt in takes:
            sl = slice(p0, p0 + cnt)
            idx16[c, t, :cnt] = (sc[sl] - bb * BLOCK).astype(np.int16)
            dl_a[c, t, :cnt] = (dc[sl] - goff).astype(NPF)
            nm_a[c, t, :cnt] = wc[sl].astype(NPF)

    # groups: runs of <=TG tiles, same block, within one stripe, and with
    # dst-window span <= GSPAN (one PSUM bank accumulates the whole group)
    groups = []
    t = 0
    while t < T:
        b = sched[t][0]
        bend = t
        while bend < T and sched[bend][0] == b:
            bend += 1
        while t < bend:
            nt = min(TG, bend - t, (t // SM + 1) * SM - t)
            while nt > 1 and sched[t + nt - 1][1] - sched[t][1] > GSPAN - W:
                nt -= 1
            groups.append((t, nt, b))
            t += nt

    NC128 = ncp // 128
    per_core = []
    for c in range(NCORES):
        idxw = idx16[c].reshape(Tm, 8, 16).transpose(2, 0, 1).copy()
        dlw = dl_a[c].transpose(1, 0).copy()
        nmw = nm_a[c].transpose(1, 0).copy()
        vd = np.zeros((128, NC128), dtype=NPF)
        lo = c * n_core
        cnt = max(0, min(n - lo, n_core))
        if cnt > 0:
            col = np.zeros(NC128 * 128, dtype=NPF)
            col[:cnt] = 1.0 / deg[lo:lo + cnt]
            vd[:, :] = col.reshape(NC128, 128).T
        per_core.append({"idxw": idxw, "dl": dlw, "nm": nmw, "vdeg": vd})

    return {
        "per_core": per_core, "n": n, "n_core": n_core, "ncp": ncp,
        "B": B, "BLOCK": BLOCK, "T": T, "sched": tuple(sched),
        "groups": groups, "deg": deg,
    }


def run_conv(meta, x_full, Wmat):
    """x_full: [n, cin] fp32 (full, unpadded); Wmat: [cin, 128].
    Returns y_out [n, 128] fp32 = GCN aggregation @ W (no bias)."""
    cin = x_full.shape[1]
    nc = build_conv_kernel(cin, meta["B"], meta["BLOCK"], meta["ncp"],
                           meta["sched"], meta["groups"])
    n, n_core, ncp = meta["n"], meta["n_core"], meta["ncp"]
    xb = np.zeros((meta["B"] * meta["BLOCK"], cin), dtype=NPF)
    xb[:n] = x_full
    Wf = np.ascontiguousarray(Wmat.astype(NPF))
    in_maps = []
    for c in range(NCORES):
        pc = meta["per_core"][c]
        lo = c * n_core
        xsc = np.zeros((ncp, cin), dtype=NPF)
        cnt = max(0, min(n - lo, n_core))
        if cnt > 0:
            xsc[:cnt] = x_full[lo:lo + cnt]
        in_maps.append({
            "xb": xb, "xs": xsc, "idxw": pc["idxw"], "dl": pc["dl"],
            "nm": pc["nm"], "vdeg": pc["vdeg"], "wmat": Wf,
            "iota": IOTA, "iota3": IOTA3, "zro": ZROS, "pidx": PIDX,
        })
    trace = bool(int(os.environ.get("GNN_TRACE", "0")))
    res = bass_utils.run_bass_kernel_spmd(
        nc, in_maps, core_ids=list(range(NCORES)), trace=trace)
    if res.exec_time_ns is not None:
        EXEC_NS.append(res.exec_time_ns)
    outs = [r["outT"] for r in res.results]
    y = np.concatenate([o.T for o in outs], axis=0)  # [8*ncp, 128]
    if ncp != n_core:
        y = y.reshape(NCORES, ncp, 128)[:, :n_core].reshape(-1, 128)
    return np.ascontiguousarray(y[:n])


# ---------------------------------------------------------------------------
# Host reference pieces (numpy, matching reference.py semantics)
# ---------------------------------------------------------------------------
def bn_relu(x, g, beta):
    m = x.mean(axis=0, dtype=np.float64).astype(NPF)
    v = ((x - m) ** 2).mean(axis=0, dtype=np.float64).astype(NPF)
    out = (x - m) * (1.0 / np.sqrt(v + EPS)) * g + beta
    return np.maximum(out, 0.0).astype(NPF)


def topk_host(score, k):
    # match jax.lax.top_k: descending values, ties -> lower index first
    idx = np.argsort(-score, kind="stable")[:k]
    return idx.astype(np.int64)


def kernel(x, edge_index, in_W, in_b, dn_W, dn_b, dn_g, dn_beta, pool_w,
           bot_W, bot_b, up_W, up_b, up_g, up_beta, out_W, out_b):
    x = np.asarray(x, dtype=NPF)
    src = np.asarray(edge_index[0], dtype=np.int64)
    dst = np.asarray(edge_index[1], dtype=np.int64)
    n = x.shape[0]

    meta0 = build_level_meta(src, dst, n)

    # in conv
    x = run_conv(meta0, x, np.asarray(in_W)) + np.asarray(in_b, dtype=NPF)

    xs, stack = [], []
    cur_src, cur_dst, cur_n, cur_meta = src, dst, n, meta0
    for i in range(DEPTH):
        x = run_conv(cur_meta, x, np.asarray(dn_W[i])) + \
            np.asarray(dn_b[i], dtype=NPF)
        x = bn_relu(x, np.asarray(dn_g[i], dtype=NPF),
                    np.asarray(dn_beta[i], dtype=NPF))
        xs.append(x)
        k = int(RATIO * cur_n)
        w = np.asarray(pool_w[i], dtype=NPF)
        score = np.tanh(x @ w / np.sqrt((w * w).sum()))
        idx = topk_host(score, k)
        new_id = np.zeros(cur_n, dtype=np.int64)
        new_id[idx] = np.arange(k)
        kept = np.zeros(cur_n, dtype=bool)
        kept[idx] = True
        emask = kept[cur_src] & kept[cur_dst]
        stack.append((cur_meta, idx, cur_n))
        cur_src = new_id[cur_src[emask]]
        cur_dst = new_id[cur_dst[emask]]
        cur_n = k
        x = x[idx]
        cur_meta = build_level_meta(cur_src, cur_dst, cur_n)

    # bottleneck
    x = run_conv(cur_meta, x, np.asarray(bot_W)) + \
        np.asarray(bot_b, dtype=NPF)
    x = np.maximum(x, 0.0)

    for i in range(DEPTH):
        p_meta, idx, pn = stack[DEPTH - 1 - i]
        xf = np.zeros((pn, x.shape[1]), dtype=NPF)
        xf[idx] = x
        x = np.concatenate([xf, xs[DEPTH - 1 - i]], axis=1)
        x = run_conv(p_meta, x, np.asarray(up_W[i])) + \
            np.asarray(up_b[i], dtype=NPF)
        x = bn_relu(x, np.asarray(up_g[i], dtype=NPF),
                    np.asarray(up_beta[i], dtype=NPF))
        cur_meta = p_meta

    out = run_conv(cur_meta, x, np.asarray(out_W)) + \
        np.asarray(out_b, dtype=NPF)
    return out.astype(np.float32)


# revision 14
# speedup vs baseline: 5.5800x; 1.1964x over previous
"""GraphUNet Trainium kernel (v2).

Architecture: 9 GCN convs (in, dn0, dn1, dn2, bottom, up0, up1, up2, out)
with top-k pooling / unpooling and batch-norm+relu between convs.

Device per conv: edge aggregation y[d] = sum_{e: dst=d} norm_e * x[src_e]
 + self-loop x[d]/deg[d], then out = (y @ W).T via:
  - dma_gather of x rows (fp32, 4 SWDGE queues round-robin) in tiles of
    128 edges packed by a joint-greedy sliding-window schedule (shared
    across the 8 SPMD cores; per-core slot data fills the tiles),
  - fp16 one-hot matmul: psum[128c x 64] += x_rows^T @ onehot(dst)*norm,
  - self-loop term via sequential shard stream + diagonal matmul
    (no gather descriptors; also serves as the y initializer),
  - final fp32 matmul streaming y through W.

Host (numpy): sharding/metadata build, top-k pools, edge relabeling,
degree/norm precompute, batch-norm, relu, bias, concat.

Sharding: dst-node ranges across 8 cores; x replicated to all cores
(graph/data parallel; halo exchange realized as full replication of the
per-conv feature table, re-staged by the host between launches).
"""

import math
import os
import sys

import numpy as np

sys.path.insert(0, "/opt/trn_rl_repo")

import concourse.bass as bass  # noqa: E402,F401
import concourse.bacc as bacc  # noqa: E402
import concourse.tile as tile  # noqa: E402
from concourse import mybir  # noqa: E402
from concourse import bass_utils  # noqa: E402

# ---- problem constants (hardcoded per task statement) ----
C_IN = 128
H = 128
DEPTH = 3
RATIO = 0.5
EPS = 1e-5
NCORES = 8
W = 64            # one-hot dst window width
TG = 8            # tiles per gather call (1024 idxs = SWDGE ring limit)
SM = 768          # tiles per metadata stripe (multiple of TG)
NQ = 4            # SWDGE queues (desc-gen parallelism)
IDXMAX = 32768    # int16 gather index reach

FP = mybir.dt.float32
FH = mybir.dt.float16
NPF = np.float32
NPH = np.float16

IOTA = np.broadcast_to(np.arange(128, dtype=NPH), (128, 128)).copy()
PIDX = np.arange(128, dtype=NPF).reshape(128, 1).copy()
IOTA3 = np.broadcast_to(np.arange(W, dtype=NPF), (128, TG, W)).reshape(
    128, TG * W).copy()
ZROS = np.zeros((128, 512), dtype=NPH)
GSPAN = 512       # group psum window width (one PSUM bank)

EXEC_NS = []  # accumulated HW exec times when tracing enabled


# ---------------------------------------------------------------------------
# Bass kernel builder (one conv shape + baked tile schedule).
# ---------------------------------------------------------------------------
_KERNEL_CACHE = {}


def build_conv_kernel(cin, B, BLOCK, ncp, sched, groups):
    """sched: list of (block, goff) per tile; groups: list of (t0, nt, b).

    DRAM inputs (per core):
      xb    fp32 [B*BLOCK, cin]  full (padded) node features
      xs    fp32 [ncp, cin]      this core's dst-shard rows (padded)
      idxw  int16 [16, T, 8]     wrapped per-tile local src indices
      dl    fp32 [128, T]        per-slot dst offset within window (0..W-1)
      nm    fp32 [128, T]        per-slot edge norm (0 for padding)
      vdeg  fp32 [128, NC128]    1/deg for shard nodes (wrapped by chunk)
      wmat  fp32 [cin, 128]      weight
      iota  fp16 [128, 128]      col j = j in every partition
      pidx  fp32 [128, 1]        partition index column
    DRAM output:
      outT  fp32 [128, ncp]      (= (y @ W).T for this core's shard)
    """
    T = len(sched)
    key = (cin, B, BLOCK, ncp, hash(tuple(sched)))
    if key in _KERNEL_CACHE:
        return _KERNEL_CACHE[key]

    KC = cin // 128
    NC128 = ncp // 128
    # fp16 x staging: halves gather bytes (256B rows measured faster than
    # 512B fp32 on HW), deletes the cast stage; gathered values were being
    # cast to fp16 for the matmul anyway.
    half = True
    XDT = FH if half else FP

    nc = bacc.Bacc("TRN2", target_bir_lowering=False, debug=False,
                   num_devices=NCORES, num_swdge_queues=NQ,
                   dynamic_dma_scratch_size=16384)

    xb = nc.dram_tensor("xb", [B * BLOCK, cin], XDT, kind="ExternalInput").ap()
    xs = nc.dram_tensor("xs", [ncp, cin], XDT, kind="ExternalInput").ap()
    Tm = max(T, 1)  # zero-size DRAM tensors are awkward; keep >=1
    idxw = nc.dram_tensor("idxw", [16, Tm, 8], mybir.dt.int16,
                          kind="ExternalInput").ap()
    dl = nc.dram_tensor("dl", [128, Tm], FP, kind="ExternalInput").ap()
    nm = nc.dram_tensor("nm", [128, Tm], FP, kind="ExternalInput").ap()
    vdeg = nc.dram_tensor("vdeg", [128, NC128], FP, kind="ExternalInput").ap()
    wmat = nc.dram_tensor("wmat", [cin, 128], FP, kind="ExternalInput").ap()
    iota = nc.dram_tensor("iota", [128, 128], FH, kind="ExternalInput").ap()
    iota3 = nc.dram_tensor("iota3", [128, TG * W], FP,
                           kind="ExternalInput").ap()
    zro = nc.dram_tensor("zro", [128, GSPAN], FH, kind="ExternalInput").ap()
    pidx = nc.dram_tensor("pidx", [128, 1], FP, kind="ExternalInput").ap()
    outT = nc.dram_tensor("outT", [128, ncp], FP, kind="ExternalOutput").ap()

    with tile.TileContext(nc) as tc:
        with (
            tc.tile_pool(name="const", bufs=1) as constp,
            tc.tile_pool(name="y", bufs=1) as yp,
            tc.tile_pool(name="meta", bufs=2) as metap,
            tc.tile_pool(name="xsl", bufs=3) as xslp,
            tc.tile_pool(name="gath", bufs=5 if cin == 128 else 3) as gathp,
            tc.tile_pool(name="ghalf", bufs=3) as ghp,
            tc.tile_pool(name="nh", bufs=4) as nhp,
            tc.tile_pool(name="eq", bufs=3) as eqp,
            tc.tile_pool(name="pg", bufs=4 if cin == 128 else 2,
                         space="PSUM") as pgp,
            tc.tile_pool(name="sf", bufs=2, space="PSUM") as sfp,
            tc.tile_pool(name="fin", bufs=2, space="PSUM") as finp,
            tc.tile_pool(name="ob", bufs=2) as obp,
        ):
            iota_t = constp.tile([128, 128], FH)
            nc.sync.dma_start(iota_t[:], iota[:])
            iota3_t = constp.tile([128, TG, W], FP)
            nc.sync.dma_start(iota3_t[:], iota3[:])
            zro_t = constp.tile([128, GSPAN], FH)
            nc.sync.dma_start(zro_t[:], zro[:])
            pidx_t = constp.tile([128, 1], FP)
            nc.sync.dma_start(pidx_t[:], pidx[:])
            vdeg_t = constp.tile([128, NC128], FP)
            nc.sync.dma_start(vdeg_t[:], vdeg[:])
            w_t = []
            for kc in range(KC):
                wt = constp.tile([128, 128], FP, name=f"wt{kc}")
                nc.sync.dma_start(wt[:], wmat[kc * 128:(kc + 1) * 128, :])
                w_t.append(wt)
            y_t = [yp.tile([128, ncp], FP, name=f"yt{kc}") for kc in range(KC)]

            regs = {}
            for nt in sorted({g[1] for g in groups}):
                regs[nt] = nc.gpsimd.to_reg(nt * 128)

            # ---- self-loop pass: y[:, chunk] = (xs_chunk^T) @ diag(1/deg)
            # (initializes y; pad chunks have vdeg=0 so y pad cols are 0)
            for k in range(NC128):
                xst = xslp.tile([128, cin], XDT)
                nc.sync.dma_start(xst[:], xs[k * 128:(k + 1) * 128, :])
                if half:
                    xsh = xst
                else:
                    xsh = xslp.tile([128, cin], FH)
                    nc.scalar.copy(xsh[:], xst[:])
                Dt = nhp.tile([128, 128], FH)
                nc.vector.tensor_scalar(
                    Dt[:], iota_t[:], pidx_t[:, 0:1], vdeg_t[:, k:k + 1],
                    mybir.AluOpType.is_equal, mybir.AluOpType.mult)
                for kc in range(KC):
                    sf_t = sfp.tile([128, 128], mybir.dt.float32)
                    nc.tensor.matmul(sf_t[:], xsh[:, kc * 128:(kc + 1) * 128],
                                     Dt[:], start=True, stop=True)
                    nc.scalar.copy(y_t[kc][:, k * 128:(k + 1) * 128], sf_t[:])

            # ---- edge tiles (gather + batched one-hot + group-psum accum)
            nstripe = (T + SM - 1) // SM if T > 0 else 0
            stripe_tiles = {}

            def load_stripe(s):
                st0 = s * SM
                stn = min(SM, T - st0)
                i_sb = metap.tile([128, SM, 8], mybir.dt.int16)
                for k in range(8):
                    nc.sync.dma_start(i_sb[16 * k:16 * (k + 1), 0:stn, :],
                                      idxw[:, st0:st0 + stn, :])
                d_sb = metap.tile([128, SM], FP)
                nc.sync.dma_start(d_sb[:, 0:stn], dl[:, st0:st0 + stn])
                n_sb = metap.tile([128, SM], FP)
                nc.sync.dma_start(n_sb[:, 0:stn], nm[:, st0:st0 + stn])
                stripe_tiles[s] = (i_sb, d_sb, n_sb)

            if nstripe > 0:
                load_stripe(0)
            for gi, (t0, nt, b) in enumerate(groups):
                s0 = t0 // SM
                if s0 + 1 < nstripe and s0 + 1 not in stripe_tiles \
                        and t0 - s0 * SM >= SM // 2:
                    load_stripe(s0 + 1)
                idx_sb, dl_sb, nm_sb = stripe_tiles[s0]
                lo = t0 - s0 * SM
                gbase = sched[t0][1]
                gw = sched[t0 + nt - 1][1] + W - gbase
                gt = gathp.tile([128, TG, cin], XDT)
                nc.gpsimd.dma_gather(
                    gt[:, 0:nt, :], xb[b * BLOCK:(b + 1) * BLOCK, :],
                    idx_sb[:, lo:lo + nt, :], nt * 128, regs[nt], cin,
                    queue_num=gi % NQ)
                if half:
                    ght = gt
                else:
                    ght = ghp.tile([128, TG, cin], FH)
                    nc.scalar.copy(ght[:, 0:nt, :], gt[:, 0:nt, :])
                eq_t = eqp.tile([128, TG, W], FP)
                dlb = dl_sb[:, lo:lo + nt].unsqueeze(2).broadcast_to(
                    (128, nt, W))
                nmb = nm_sb[:, lo:lo + nt].unsqueeze(2).broadcast_to(
                    (128, nt, W))
                nc.vector.tensor_tensor(eq_t[:, 0:nt, :], iota3_t[:, 0:nt, :],
                                        dlb, mybir.AluOpType.is_equal)
                nh_g = nhp.tile([128, TG, W], FH)
                nc.vector.tensor_tensor(nh_g[:, 0:nt, :], eq_t[:, 0:nt, :],
                                        nmb, mybir.AluOpType.mult)
                pg_t = [pgp.tile([128, GSPAN], mybir.dt.float32,
                                 name=f"pg{kc}") for kc in range(KC)]
                for kc in range(KC):
                    nc.tensor.matmul(pg_t[kc][:, 0:gw], iota_t[:],
                                     zro_t[:, 0:gw], start=True, stop=False,
                                     skip_group_check=True)
                for jt in range(nt):
                    po = sched[t0 + jt][1] - gbase
                    for kc in range(KC):
                        nc.tensor.matmul(
                            pg_t[kc][:, po:po + W],
                            ght[:, jt, kc * 128:(kc + 1) * 128],
                            nh_g[:, jt, :], start=False, stop=(jt == nt - 1),
                            skip_group_check=True)
                for kc in range(KC):
                    nc.vector.tensor_add(
                        y_t[kc][:, gbase:gbase + gw],
                        y_t[kc][:, gbase:gbase + gw], pg_t[kc][:, 0:gw])

            # ---- final: outT = sum_kc W[kc].T @ y[kc]
            FC = 512
            q0 = 0
            while q0 < ncp:
                fq = min(FC, ncp - q0)
                fp_t = finp.tile([128, fq], mybir.dt.float32)
                for kc in range(KC):
                    nc.tensor.matmul(fp_t[:], w_t[kc][:],
                                     y_t[kc][:, q0:q0 + fq],
                                     start=(kc == 0), stop=(kc == KC - 1))
                ob_t = obp.tile([128, fq], FP)
                nc.scalar.copy(ob_t[:], fp_t[:])
                nc.sync.dma_start(outT[:, q0:q0 + fq], ob_t[:])
                q0 += fq

    nc.compile()
    _KERNEL_CACHE[key] = nc
    return nc


# ---------------------------------------------------------------------------
# Host-side metadata build for one graph level
# ---------------------------------------------------------------------------
def build_level_meta(src, dst, n):
    """src/dst: active edges (compacted, relabeled) int64 arrays; n nodes.

    Builds the SPMD-shared joint-greedy tile schedule + per-core slot data.
    """
    n_core = (n + NCORES - 1) // NCORES
    ncp = max(128, ((n_core + 127) // 128) * 128)
    B = max(1, (n + IDXMAX - 1) // IDXMAX)
    BLOCK = ((n + B - 1) // B + 7) // 8 * 8
    assert BLOCK <= IDXMAX

    deg = (np.bincount(dst, minlength=n) + 1.0).astype(NPF)
    dinv = (1.0 / np.sqrt(deg)).astype(NPF)
    enorm = (dinv[src] * dinv[dst]).astype(NPF)

    core = dst // n_core
    dloc = (dst - core * n_core).astype(np.int64)
    blk = src // BLOCK

    order = np.lexsort((dloc, blk, core))
    sc = src[order]
    dc = dloc[order]
    wc = enorm[order]
    keys = (core[order] * B + blk[order]).astype(np.int64)

    # segment bounds per (core, block)
    bounds = np.searchsorted(keys, np.arange(NCORES * B + 1))

    sched = []          # (block, goff)
    tile_take = []      # per tile: list of (core, pos, cnt)
    for b in range(B):
        pos = [int(bounds[c * B + b]) for c in range(NCORES)]
        hi = [int(bounds[c * B + b + 1]) for c in range(NCORES)]
        while True:
            base = None
            for c in range(NCORES):
                if pos[c] < hi[c]:
                    v = int(dc[pos[c]])
                    if base is None or v < base:
                        base = v
            if base is None:
                break
            base = min(base, ncp - W)
            takes = []
            for c in range(NCORES):
                if pos[c] < hi[c]:
                    e = pos[c] + int(np.searchsorted(
                        dc[pos[c]:hi[c]], base + W, side="left"))
                    cnt = min(128, e - pos[c])
                    if cnt > 0:
                        takes.append((c, pos[c], cnt))
                        pos[c] += cnt
            sched.append((b, base))
            tile_take.append(takes)

    T = len(sched)
    Tm = max(T, 1)
    idx16 = np.zeros((NCORES, Tm, 128), dtype=np.int16)
    dl_a = np.zeros((NCORES, Tm, 128), dtype=NPF)
    nm_a = np.zeros((NCORES, Tm, 128), dtype=NPF)
    for t, takes in enumerate(tile_take):
        bb, goff = sched[t]
        for c, p0, cnt in takes:
            sl = slice(p0, p0 + cnt)
            idx16[c, t, :cnt] = (sc[sl] - bb * BLOCK).astype(np.int16)
            dl_a[c, t, :cnt] = (dc[sl] - goff).astype(NPF)
            nm_a[c, t, :cnt] = wc[sl].astype(NPF)

    # groups: runs of <=TG tiles, same block, within one stripe, and with
    # dst-window span <= GSPAN (one PSUM bank accumulates the whole group)
    groups = []
    t = 0
    while t < T:
        b = sched[t][0]
        bend = t
        while bend < T and sched[bend][0] == b:
            bend += 1
        while t < bend:
            nt = min(TG, bend - t, (t // SM + 1) * SM - t)
            while nt > 1 and sched[t + nt - 1][1] - sched[t][1] > GSPAN - W:
                nt -= 1
            groups.append((t, nt, b))
            t += nt

    NC128 = ncp // 128
    per_core = []
    for c in range(NCORES):
        idxw = idx16[c].reshape(Tm, 8, 16).transpose(2, 0, 1).copy()
        dlw = dl_a[c].transpose(1, 0).copy()
        nmw = nm_a[c].transpose(1, 0).copy()
        vd = np.zeros((128, NC128), dtype=NPF)
        lo = c * n_core
        cnt = max(0, min(n - lo, n_core))
        if cnt > 0:
            col = np.zeros(NC128 * 128, dtype=NPF)
            col[:cnt] = 1.0 / deg[lo:lo + cnt]
            vd[:, :] = col.reshape(NC128, 128).T
        per_core.append({"idxw": idxw, "dl": dlw, "nm": nmw, "vdeg": vd})

    return {
        "per_core": per_core, "n": n, "n_core": n_core, "ncp": ncp,
        "B": B, "BLOCK": BLOCK, "T": T, "sched": tuple(sched),
        "groups": groups, "deg": deg,
    }


def run_conv(meta, x_full, Wmat):
    """x_full: [n, cin] fp32 (full, unpadded); Wmat: [cin, 128].
    Returns y_out [n, 128] fp32 = GCN aggregation @ W (no bias)."""
    cin = x_full.shape[1]
    nc = build_conv_kernel(cin, meta["B"], meta["BLOCK"], meta["ncp"],
                           meta["sched"], meta["groups"])
    n, n_core, ncp = meta["n"], meta["n_core"], meta["ncp"]
    xb = np.zeros((meta["B"] * meta["BLOCK"], cin), dtype=NPH)
    xb[:n] = x_full
    Wf = np.ascontiguousarray(Wmat.astype(NPF))
    in_maps = []
    for c in range(NCORES):
        pc = meta["per_core"][c]
        lo = c * n_core
        xsc = np.zeros((ncp, cin), dtype=NPH)
        cnt = max(0, min(n - lo, n_core))
        if cnt > 0:
            xsc[:cnt] = x_full[lo:lo + cnt]
        in_maps.append({
            "xb": xb, "xs": xsc, "idxw": pc["idxw"], "dl": pc["dl"],
            "nm": pc["nm"], "vdeg": pc["vdeg"], "wmat": Wf,
            "iota": IOTA, "iota3": IOTA3, "zro": ZROS, "pidx": PIDX,
        })
    trace = bool(int(os.environ.get("GNN_TRACE", "0")))
    res = bass_utils.run_bass_kernel_spmd(
        nc, in_maps, core_ids=list(range(NCORES)), trace=trace)
    if res.exec_time_ns is not None:
        EXEC_NS.append(res.exec_time_ns)
    outs = [r["outT"] for r in res.results]
    y = np.concatenate([o.T for o in outs], axis=0)  # [8*ncp, 128]
    if ncp != n_core:
        y = y.reshape(NCORES, ncp, 128)[:, :n_core].reshape(-1, 128)
    return np.ascontiguousarray(y[:n])


# ---------------------------------------------------------------------------
# Host reference pieces (numpy, matching reference.py semantics)
# ---------------------------------------------------------------------------
def bn_relu(x, g, beta):
    m = x.mean(axis=0, dtype=np.float64).astype(NPF)
    v = ((x - m) ** 2).mean(axis=0, dtype=np.float64).astype(NPF)
    out = (x - m) * (1.0 / np.sqrt(v + EPS)) * g + beta
    return np.maximum(out, 0.0).astype(NPF)


def topk_host(score, k):
    # match jax.lax.top_k: descending values, ties -> lower index first
    idx = np.argsort(-score, kind="stable")[:k]
    return idx.astype(np.int64)


def kernel(x, edge_index, in_W, in_b, dn_W, dn_b, dn_g, dn_beta, pool_w,
           bot_W, bot_b, up_W, up_b, up_g, up_beta, out_W, out_b):
    x = np.asarray(x, dtype=NPF)
    src = np.asarray(edge_index[0], dtype=np.int64)
    dst = np.asarray(edge_index[1], dtype=np.int64)
    n = x.shape[0]

    meta0 = build_level_meta(src, dst, n)

    # in conv
    x = run_conv(meta0, x, np.asarray(in_W)) + np.asarray(in_b, dtype=NPF)

    xs, stack = [], []
    cur_src, cur_dst, cur_n, cur_meta = src, dst, n, meta0
    for i in range(DEPTH):
        x = run_conv(cur_meta, x, np.asarray(dn_W[i])) + \
            np.asarray(dn_b[i], dtype=NPF)
        x = bn_relu(x, np.asarray(dn_g[i], dtype=NPF),
                    np.asarray(dn_beta[i], dtype=NPF))
        xs.append(x)
        k = int(RATIO * cur_n)
        w = np.asarray(pool_w[i], dtype=NPF)
        score = np.tanh(x @ w / np.sqrt((w * w).sum()))
        idx = topk_host(score, k)
        new_id = np.zeros(cur_n, dtype=np.int64)
        new_id[idx] = np.arange(k)
        kept = np.zeros(cur_n, dtype=bool)
        kept[idx] = True
        emask = kept[cur_src] & kept[cur_dst]
        stack.append((cur_meta, idx, cur_n))
        cur_src = new_id[cur_src[emask]]
        cur_dst = new_id[cur_dst[emask]]
        cur_n = k
        x = x[idx]
        cur_meta = build_level_meta(cur_src, cur_dst, cur_n)

    # bottleneck
    x = run_conv(cur_meta, x, np.asarray(bot_W)) + \
        np.asarray(bot_b, dtype=NPF)
    x = np.maximum(x, 0.0)

    for i in range(DEPTH):
        p_meta, idx, pn = stack[DEPTH - 1 - i]
        xf = np.zeros((pn, x.shape[1]), dtype=NPF)
        xf[idx] = x
        x = np.concatenate([xf, xs[DEPTH - 1 - i]], axis=1)
        x = run_conv(p_meta, x, np.asarray(up_W[i])) + \
            np.asarray(up_b[i], dtype=NPF)
        x = bn_relu(x, np.asarray(up_g[i], dtype=NPF),
                    np.asarray(up_beta[i], dtype=NPF))
        cur_meta = p_meta

    out = run_conv(cur_meta, x, np.asarray(out_W)) + \
        np.asarray(out_b, dtype=NPF)
    return out.astype(np.float32)


# revision 15
# speedup vs baseline: 5.8019x; 1.0398x over previous
"""GraphUNet Trainium kernel (v2).

Architecture: 9 GCN convs (in, dn0, dn1, dn2, bottom, up0, up1, up2, out)
with top-k pooling / unpooling and batch-norm+relu between convs.

Device per conv: edge aggregation y[d] = sum_{e: dst=d} norm_e * x[src_e]
 + self-loop x[d]/deg[d], then out = (y @ W).T via:
  - dma_gather of x rows (fp32, 4 SWDGE queues round-robin) in tiles of
    128 edges packed by a joint-greedy sliding-window schedule (shared
    across the 8 SPMD cores; per-core slot data fills the tiles),
  - fp16 one-hot matmul: psum[128c x 64] += x_rows^T @ onehot(dst)*norm,
  - self-loop term via sequential shard stream + diagonal matmul
    (no gather descriptors; also serves as the y initializer),
  - final fp32 matmul streaming y through W.

Host (numpy): sharding/metadata build, top-k pools, edge relabeling,
degree/norm precompute, batch-norm, relu, bias, concat.

Sharding: dst-node ranges across 8 cores; x replicated to all cores
(graph/data parallel; halo exchange realized as full replication of the
per-conv feature table, re-staged by the host between launches).
"""

import math
import os
import sys

import numpy as np

sys.path.insert(0, "/opt/trn_rl_repo")

import concourse.bass as bass  # noqa: E402,F401
import concourse.bacc as bacc  # noqa: E402
import concourse.tile as tile  # noqa: E402
from concourse import mybir  # noqa: E402
from concourse import bass_utils  # noqa: E402

# ---- problem constants (hardcoded per task statement) ----
C_IN = 128
H = 128
DEPTH = 3
RATIO = 0.5
EPS = 1e-5
NCORES = 8
W = 64            # one-hot dst window width
TG = 8            # tiles per gather call (1024 idxs = SWDGE ring limit)
SM = 768          # tiles per metadata stripe (multiple of TG)
NQ = 4            # SWDGE queues (desc-gen parallelism)
IDXMAX = 32768    # int16 gather index reach

FP = mybir.dt.float32
FH = mybir.dt.float16
NPF = np.float32
NPH = np.float16

IOTA = np.broadcast_to(np.arange(128, dtype=NPH), (128, 128)).copy()
PIDX = np.arange(128, dtype=NPF).reshape(128, 1).copy()
IOTA3 = np.broadcast_to(np.arange(W, dtype=NPF), (128, TG, W)).reshape(
    128, TG * W).copy()
ZROS = np.zeros((128, 512), dtype=NPH)
GSPAN = 512       # group psum window width (one PSUM bank)

EXEC_NS = []  # accumulated HW exec times when tracing enabled


# ---------------------------------------------------------------------------
# Bass kernel builder (one conv shape + baked tile schedule).
# ---------------------------------------------------------------------------
_KERNEL_CACHE = {}


def build_conv_kernel(cin, B, BLOCK, ncp, sched, groups):
    """sched: list of (block, goff) per tile; groups: list of (t0, nt, b).

    DRAM inputs (per core):
      xb    fp32 [B*BLOCK, cin]  full (padded) node features
      xs    fp32 [ncp, cin]      this core's dst-shard rows (padded)
      idxw  int16 [16, T, 8]     wrapped per-tile local src indices
      dl    fp32 [128, T]        per-slot dst offset within window (0..W-1)
      nm    fp32 [128, T]        per-slot edge norm (0 for padding)
      vdeg  fp32 [128, NC128]    1/deg for shard nodes (wrapped by chunk)
      wmat  fp32 [cin, 128]      weight
      iota  fp16 [128, 128]      col j = j in every partition
      pidx  fp32 [128, 1]        partition index column
    DRAM output:
      outT  fp32 [128, ncp]      (= (y @ W).T for this core's shard)
    """
    T = len(sched)
    key = (cin, B, BLOCK, ncp, hash(tuple(sched)))
    if key in _KERNEL_CACHE:
        return _KERNEL_CACHE[key]

    KC = cin // 128
    NC128 = ncp // 128
    # fp16 x staging: halves gather bytes (256B rows measured faster than
    # 512B fp32 on HW), deletes the cast stage; gathered values were being
    # cast to fp16 for the matmul anyway.
    half = True
    XDT = FH if half else FP

    nc = bacc.Bacc("TRN2", target_bir_lowering=False, debug=False,
                   num_devices=NCORES, num_swdge_queues=NQ,
                   dynamic_dma_scratch_size=16384)

    xb = nc.dram_tensor("xb", [B * BLOCK, cin], XDT, kind="ExternalInput").ap()
    xs = nc.dram_tensor("xs", [ncp, cin], XDT, kind="ExternalInput").ap()
    Tm = max(T, 1)  # zero-size DRAM tensors are awkward; keep >=1
    idxw = nc.dram_tensor("idxw", [16, Tm, 8], mybir.dt.int16,
                          kind="ExternalInput").ap()
    dl = nc.dram_tensor("dl", [128, Tm], FP, kind="ExternalInput").ap()
    nm = nc.dram_tensor("nm", [128, Tm], FP, kind="ExternalInput").ap()
    vdeg = nc.dram_tensor("vdeg", [128, NC128], FP, kind="ExternalInput").ap()
    wmat = nc.dram_tensor("wmat", [cin, 128], FP, kind="ExternalInput").ap()
    iota = nc.dram_tensor("iota", [128, 128], FH, kind="ExternalInput").ap()
    iota3 = nc.dram_tensor("iota3", [128, TG * W], FP,
                           kind="ExternalInput").ap()
    zro = nc.dram_tensor("zro", [128, GSPAN], FH, kind="ExternalInput").ap()
    pidx = nc.dram_tensor("pidx", [128, 1], FP, kind="ExternalInput").ap()
    outT = nc.dram_tensor("outT", [128, ncp], FP, kind="ExternalOutput").ap()

    with tile.TileContext(nc) as tc:
        with (
            tc.tile_pool(name="const", bufs=1) as constp,
            tc.tile_pool(name="y", bufs=1) as yp,
            tc.tile_pool(name="meta", bufs=2) as metap,
            tc.tile_pool(name="xsl", bufs=3) as xslp,
            tc.tile_pool(name="gath", bufs=5 if cin == 128 else 4) as gathp,
            tc.tile_pool(name="ghalf", bufs=3) as ghp,
            tc.tile_pool(name="nh", bufs=4) as nhp,
            tc.tile_pool(name="eq", bufs=3) as eqp,
            tc.tile_pool(name="pg", bufs=6 if cin == 128 else 3,
                         space="PSUM") as pgp,
            tc.tile_pool(name="sf", bufs=1, space="PSUM") as sfp,
            tc.tile_pool(name="fin", bufs=1, space="PSUM") as finp,
            tc.tile_pool(name="ob", bufs=2) as obp,
        ):
            iota_t = constp.tile([128, 128], FH)
            nc.sync.dma_start(iota_t[:], iota[:])
            iota3_t = constp.tile([128, TG, W], FP)
            nc.sync.dma_start(iota3_t[:], iota3[:])
            zro_t = constp.tile([128, GSPAN], FH)
            nc.sync.dma_start(zro_t[:], zro[:])
            pidx_t = constp.tile([128, 1], FP)
            nc.sync.dma_start(pidx_t[:], pidx[:])
            vdeg_t = constp.tile([128, NC128], FP)
            nc.sync.dma_start(vdeg_t[:], vdeg[:])
            w_t = []
            for kc in range(KC):
                wt = constp.tile([128, 128], FP, name=f"wt{kc}")
                nc.sync.dma_start(wt[:], wmat[kc * 128:(kc + 1) * 128, :])
                w_t.append(wt)
            y_t = [yp.tile([128, ncp], FP, name=f"yt{kc}") for kc in range(KC)]

            regs = {}
            for nt in sorted({g[1] for g in groups}):
                regs[nt] = nc.gpsimd.to_reg(nt * 128)

            # ---- self-loop pass: y[:, chunk] = (xs_chunk^T) @ diag(1/deg)
            # (initializes y; pad chunks have vdeg=0 so y pad cols are 0)
            for k in range(NC128):
                xst = xslp.tile([128, cin], XDT)
                nc.sync.dma_start(xst[:], xs[k * 128:(k + 1) * 128, :])
                if half:
                    xsh = xst
                else:
                    xsh = xslp.tile([128, cin], FH)
                    nc.scalar.copy(xsh[:], xst[:])
                Dt = nhp.tile([128, 128], FH)
                nc.vector.tensor_scalar(
                    Dt[:], iota_t[:], pidx_t[:, 0:1], vdeg_t[:, k:k + 1],
                    mybir.AluOpType.is_equal, mybir.AluOpType.mult)
                for kc in range(KC):
                    sf_t = sfp.tile([128, 128], mybir.dt.float32)
                    nc.tensor.matmul(sf_t[:], xsh[:, kc * 128:(kc + 1) * 128],
                                     Dt[:], start=True, stop=True)
                    nc.scalar.copy(y_t[kc][:, k * 128:(k + 1) * 128], sf_t[:])

            # ---- edge tiles (gather + batched one-hot + group-psum accum)
            nstripe = (T + SM - 1) // SM if T > 0 else 0
            stripe_tiles = {}

            def load_stripe(s):
                st0 = s * SM
                stn = min(SM, T - st0)
                i_sb = metap.tile([128, SM, 8], mybir.dt.int16)
                for k in range(8):
                    nc.sync.dma_start(i_sb[16 * k:16 * (k + 1), 0:stn, :],
                                      idxw[:, st0:st0 + stn, :])
                d_sb = metap.tile([128, SM], FP)
                nc.sync.dma_start(d_sb[:, 0:stn], dl[:, st0:st0 + stn])
                n_sb = metap.tile([128, SM], FP)
                nc.sync.dma_start(n_sb[:, 0:stn], nm[:, st0:st0 + stn])
                stripe_tiles[s] = (i_sb, d_sb, n_sb)

            if nstripe > 0:
                load_stripe(0)
            for gi, (t0, nt, b) in enumerate(groups):
                s0 = t0 // SM
                if s0 + 1 < nstripe and s0 + 1 not in stripe_tiles \
                        and t0 - s0 * SM >= SM // 2:
                    load_stripe(s0 + 1)
                idx_sb, dl_sb, nm_sb = stripe_tiles[s0]
                lo = t0 - s0 * SM
                gbase = sched[t0][1]
                gw = sched[t0 + nt - 1][1] + W - gbase
                gt = gathp.tile([128, TG, cin], XDT)
                nc.gpsimd.dma_gather(
                    gt[:, 0:nt, :], xb[b * BLOCK:(b + 1) * BLOCK, :],
                    idx_sb[:, lo:lo + nt, :], nt * 128, regs[nt], cin,
                    queue_num=gi % NQ)
                if half:
                    ght = gt
                else:
                    ght = ghp.tile([128, TG, cin], FH)
                    nc.scalar.copy(ght[:, 0:nt, :], gt[:, 0:nt, :])
                eq_t = eqp.tile([128, TG, W], FP)
                dlb = dl_sb[:, lo:lo + nt].unsqueeze(2).broadcast_to(
                    (128, nt, W))
                nmb = nm_sb[:, lo:lo + nt].unsqueeze(2).broadcast_to(
                    (128, nt, W))
                nc.vector.tensor_tensor(eq_t[:, 0:nt, :], iota3_t[:, 0:nt, :],
                                        dlb, mybir.AluOpType.is_equal)
                nh_g = nhp.tile([128, TG, W], FH)
                nc.vector.tensor_tensor(nh_g[:, 0:nt, :], eq_t[:, 0:nt, :],
                                        nmb, mybir.AluOpType.mult)
                pg_t = [pgp.tile([128, GSPAN], mybir.dt.float32,
                                 name=f"pg{kc}") for kc in range(KC)]
                for kc in range(KC):
                    nc.tensor.matmul(pg_t[kc][:, 0:gw], iota_t[:],
                                     zro_t[:, 0:gw], start=True, stop=False,
                                     skip_group_check=True)
                for jt in range(nt):
                    po = sched[t0 + jt][1] - gbase
                    for kc in range(KC):
                        nc.tensor.matmul(
                            pg_t[kc][:, po:po + W],
                            ght[:, jt, kc * 128:(kc + 1) * 128],
                            nh_g[:, jt, :], start=False, stop=(jt == nt - 1),
                            skip_group_check=True)
                for kc in range(KC):
                    nc.vector.tensor_add(
                        y_t[kc][:, gbase:gbase + gw],
                        y_t[kc][:, gbase:gbase + gw], pg_t[kc][:, 0:gw])

            # ---- final: outT = sum_kc W[kc].T @ y[kc]
            FC = 512
            q0 = 0
            while q0 < ncp:
                fq = min(FC, ncp - q0)
                fp_t = finp.tile([128, fq], mybir.dt.float32)
                for kc in range(KC):
                    nc.tensor.matmul(fp_t[:], w_t[kc][:],
                                     y_t[kc][:, q0:q0 + fq],
                                     start=(kc == 0), stop=(kc == KC - 1))
                ob_t = obp.tile([128, fq], FP)
                nc.scalar.copy(ob_t[:], fp_t[:])
                nc.sync.dma_start(outT[:, q0:q0 + fq], ob_t[:])
                q0 += fq

    nc.compile()
    _KERNEL_CACHE[key] = nc
    return nc


# ---------------------------------------------------------------------------
# Host-side metadata build for one graph level
# ---------------------------------------------------------------------------
def build_level_meta(src, dst, n):
    """src/dst: active edges (compacted, relabeled) int64 arrays; n nodes.

    Builds the SPMD-shared joint-greedy tile schedule + per-core slot data.
    """
    n_core = (n + NCORES - 1) // NCORES
    ncp = max(128, ((n_core + 127) // 128) * 128)
    B = max(1, (n + IDXMAX - 1) // IDXMAX)
    BLOCK = ((n + B - 1) // B + 7) // 8 * 8
    assert BLOCK <= IDXMAX

    deg = (np.bincount(dst, minlength=n) + 1.0).astype(NPF)
    dinv = (1.0 / np.sqrt(deg)).astype(NPF)
    enorm = (dinv[src] * dinv[dst]).astype(NPF)

    core = dst // n_core
    dloc = (dst - core * n_core).astype(np.int64)
    blk = src // BLOCK

    order = np.lexsort((dloc, blk, core))
    sc = src[order]
    dc = dloc[order]
    wc = enorm[order]
    keys = (core[order] * B + blk[order]).astype(np.int64)

    # segment bounds per (core, block)
    bounds = np.searchsorted(keys, np.arange(NCORES * B + 1))

    sched = []          # (block, goff)
    tile_take = []      # per tile: list of (core, pos, cnt)
    for b in range(B):
        pos = [int(bounds[c * B + b]) for c in range(NCORES)]
        hi = [int(bounds[c * B + b + 1]) for c in range(NCORES)]
        while True:
            base = None
            for c in range(NCORES):
                if pos[c] < hi[c]:
                    v = int(dc[pos[c]])
                    if base is None or v < base:
                        base = v
            if base is None:
                break
            base = min(base, ncp - W)
            takes = []
            for c in range(NCORES):
                if pos[c] < hi[c]:
                    e = pos[c] + int(np.searchsorted(
                        dc[pos[c]:hi[c]], base + W, side="left"))
                    cnt = min(128, e - pos[c])
                    if cnt > 0:
                        takes.append((c, pos[c], cnt))
                        pos[c] += cnt
            sched.append((b, base))
            tile_take.append(takes)

    T = len(sched)
    Tm = max(T, 1)
    idx16 = np.zeros((NCORES, Tm, 128), dtype=np.int16)
    dl_a = np.zeros((NCORES, Tm, 128), dtype=NPF)
    nm_a = np.zeros((NCORES, Tm, 128), dtype=NPF)
    for t, takes in enumerate(tile_take):
        bb, goff = sched[t]
        for c, p0, cnt in takes:
            sl = slice(p0, p0 + cnt)
            idx16[c, t, :cnt] = (sc[sl] - bb * BLOCK).astype(np.int16)
            dl_a[c, t, :cnt] = (dc[sl] - goff).astype(NPF)
            nm_a[c, t, :cnt] = wc[sl].astype(NPF)

    # groups: runs of <=TG tiles, same block, within one stripe, and with
    # dst-window span <= GSPAN (one PSUM bank accumulates the whole group)
    groups = []
    t = 0
    while t < T:
        b = sched[t][0]
        bend = t
        while bend < T and sched[bend][0] == b:
            bend += 1
        while t < bend:
            nt = min(TG, bend - t, (t // SM + 1) * SM - t)
            while nt > 1 and sched[t + nt - 1][1] - sched[t][1] > GSPAN - W:
                nt -= 1
            groups.append((t, nt, b))
            t += nt

    NC128 = ncp // 128
    per_core = []
    for c in range(NCORES):
        idxw = idx16[c].reshape(Tm, 8, 16).transpose(2, 0, 1).copy()
        dlw = dl_a[c].transpose(1, 0).copy()
        nmw = nm_a[c].transpose(1, 0).copy()
        vd = np.zeros((128, NC128), dtype=NPF)
        lo = c * n_core
        cnt = max(0, min(n - lo, n_core))
        if cnt > 0:
            col = np.zeros(NC128 * 128, dtype=NPF)
            col[:cnt] = 1.0 / deg[lo:lo + cnt]
            vd[:, :] = col.reshape(NC128, 128).T
        per_core.append({"idxw": idxw, "dl": dlw, "nm": nmw, "vdeg": vd})

    return {
        "per_core": per_core, "n": n, "n_core": n_core, "ncp": ncp,
        "B": B, "BLOCK": BLOCK, "T": T, "sched": tuple(sched),
        "groups": groups, "deg": deg,
    }


def run_conv(meta, x_full, Wmat):
    """x_full: [n, cin] fp32 (full, unpadded); Wmat: [cin, 128].
    Returns y_out [n, 128] fp32 = GCN aggregation @ W (no bias)."""
    cin = x_full.shape[1]
    nc = build_conv_kernel(cin, meta["B"], meta["BLOCK"], meta["ncp"],
                           meta["sched"], meta["groups"])
    n, n_core, ncp = meta["n"], meta["n_core"], meta["ncp"]
    xb = np.zeros((meta["B"] * meta["BLOCK"], cin), dtype=NPH)
    xb[:n] = x_full
    Wf = np.ascontiguousarray(Wmat.astype(NPF))
    in_maps = []
    for c in range(NCORES):
        pc = meta["per_core"][c]
        lo = c * n_core
        xsc = np.zeros((ncp, cin), dtype=NPH)
        cnt = max(0, min(n - lo, n_core))
        if cnt > 0:
            xsc[:cnt] = x_full[lo:lo + cnt]
        in_maps.append({
            "xb": xb, "xs": xsc, "idxw": pc["idxw"], "dl": pc["dl"],
            "nm": pc["nm"], "vdeg": pc["vdeg"], "wmat": Wf,
            "iota": IOTA, "iota3": IOTA3, "zro": ZROS, "pidx": PIDX,
        })
    trace = bool(int(os.environ.get("GNN_TRACE", "0")))
    res = bass_utils.run_bass_kernel_spmd(
        nc, in_maps, core_ids=list(range(NCORES)), trace=trace)
    if res.exec_time_ns is not None:
        EXEC_NS.append(res.exec_time_ns)
    outs = [r["outT"] for r in res.results]
    y = np.concatenate([o.T for o in outs], axis=0)  # [8*ncp, 128]
    if ncp != n_core:
        y = y.reshape(NCORES, ncp, 128)[:, :n_core].reshape(-1, 128)
    return np.ascontiguousarray(y[:n])


# ---------------------------------------------------------------------------
# Host reference pieces (numpy, matching reference.py semantics)
# ---------------------------------------------------------------------------
def bn_relu(x, g, beta):
    m = x.mean(axis=0, dtype=np.float64).astype(NPF)
    v = ((x - m) ** 2).mean(axis=0, dtype=np.float64).astype(NPF)
    out = (x - m) * (1.0 / np.sqrt(v + EPS)) * g + beta
    return np.maximum(out, 0.0).astype(NPF)


def topk_host(score, k):
    # match jax.lax.top_k: descending values, ties -> lower index first
    idx = np.argsort(-score, kind="stable")[:k]
    return idx.astype(np.int64)


def kernel(x, edge_index, in_W, in_b, dn_W, dn_b, dn_g, dn_beta, pool_w,
           bot_W, bot_b, up_W, up_b, up_g, up_beta, out_W, out_b):
    x = np.asarray(x, dtype=NPF)
    src = np.asarray(edge_index[0], dtype=np.int64)
    dst = np.asarray(edge_index[1], dtype=np.int64)
    n = x.shape[0]

    meta0 = build_level_meta(src, dst, n)

    # in conv
    x = run_conv(meta0, x, np.asarray(in_W)) + np.asarray(in_b, dtype=NPF)

    xs, stack = [], []
    cur_src, cur_dst, cur_n, cur_meta = src, dst, n, meta0
    for i in range(DEPTH):
        x = run_conv(cur_meta, x, np.asarray(dn_W[i])) + \
            np.asarray(dn_b[i], dtype=NPF)
        x = bn_relu(x, np.asarray(dn_g[i], dtype=NPF),
                    np.asarray(dn_beta[i], dtype=NPF))
        xs.append(x)
        k = int(RATIO * cur_n)
        w = np.asarray(pool_w[i], dtype=NPF)
        score = np.tanh(x @ w / np.sqrt((w * w).sum()))
        idx = topk_host(score, k)
        new_id = np.zeros(cur_n, dtype=np.int64)
        new_id[idx] = np.arange(k)
        kept = np.zeros(cur_n, dtype=bool)
        kept[idx] = True
        emask = kept[cur_src] & kept[cur_dst]
        stack.append((cur_meta, idx, cur_n))
        cur_src = new_id[cur_src[emask]]
        cur_dst = new_id[cur_dst[emask]]
        cur_n = k
        x = x[idx]
        cur_meta = build_level_meta(cur_src, cur_dst, cur_n)

    # bottleneck
    x = run_conv(cur_meta, x, np.asarray(bot_W)) + \
        np.asarray(bot_b, dtype=NPF)
    x = np.maximum(x, 0.0)

    for i in range(DEPTH):
        p_meta, idx, pn = stack[DEPTH - 1 - i]
        xf = np.zeros((pn, x.shape[1]), dtype=NPF)
        xf[idx] = x
        x = np.concatenate([xf, xs[DEPTH - 1 - i]], axis=1)
        x = run_conv(p_meta, x, np.asarray(up_W[i])) + \
            np.asarray(up_b[i], dtype=NPF)
        x = bn_relu(x, np.asarray(up_g[i], dtype=NPF),
                    np.asarray(up_beta[i], dtype=NPF))
        cur_meta = p_meta

    out = run_conv(cur_meta, x, np.asarray(out_W)) + \
        np.asarray(out_b, dtype=NPF)
    return out.astype(np.float32)


# revision 18
# speedup vs baseline: 5.8523x; 1.0087x over previous
"""GraphUNet Trainium kernel (v2).

Architecture: 9 GCN convs (in, dn0, dn1, dn2, bottom, up0, up1, up2, out)
with top-k pooling / unpooling and batch-norm+relu between convs.

Device per conv: edge aggregation y[d] = sum_{e: dst=d} norm_e * x[src_e]
 + self-loop x[d]/deg[d], then out = (y @ W).T via:
  - dma_gather of x rows (fp16, 4 SWDGE queues round-robin) in tiles of
    128 edges packed by a joint-greedy sliding-window schedule (shared
    across the 8 SPMD cores; per-core slot data fills the tiles),
  - fp16 one-hot matmul: psum[128c x 64] += x_rows^T @ onehot(dst)*norm,
  - self-loop term via sequential shard stream + diagonal matmul
    (no gather descriptors; also serves as the y initializer),
  - final fp32 matmul streaming y through W.

Host (numpy): sharding/metadata build, top-k pools, edge relabeling,
degree/norm precompute, batch-norm, relu, bias, concat.

Sharding: dst-node ranges across 8 cores; x replicated to all cores
(graph/data parallel; halo exchange realized as full replication of the
per-conv feature table, re-staged by the host between launches).
"""

import math
import os
import sys

import numpy as np

sys.path.insert(0, "/opt/trn_rl_repo")

import concourse.bass as bass  # noqa: E402,F401
import concourse.bacc as bacc  # noqa: E402
import concourse.tile as tile  # noqa: E402
from concourse import mybir  # noqa: E402
from concourse import bass_utils  # noqa: E402

# ---- problem constants (hardcoded per task statement) ----
C_IN = 128
H = 128
DEPTH = 3
RATIO = 0.5
EPS = 1e-5
NCORES = 8
W = 64            # one-hot dst window width
TG = 8            # tiles per gather call (1024 idxs = SWDGE ring limit)
SM = 768          # tiles per metadata stripe (multiple of TG)
NQ = 4            # SWDGE queues (desc-gen parallelism)
IDXMAX = 32768    # int16 gather index reach

FP = mybir.dt.float32
FH = mybir.dt.float16
NPF = np.float32
NPH = np.float16

IOTA = np.broadcast_to(np.arange(128, dtype=NPH), (128, 128)).copy()
PIDX = np.arange(128, dtype=NPF).reshape(128, 1).copy()
IOTA3 = np.broadcast_to(np.arange(W, dtype=NPF), (128, TG, W)).reshape(
    128, TG * W).copy()
ZROS = np.zeros((128, 512), dtype=NPH)
GSPAN = 512       # group psum window width (one PSUM bank)

EXEC_NS = []  # accumulated HW exec times when tracing enabled


# ---------------------------------------------------------------------------
# Bass kernel builder (one conv shape + baked tile schedule).
# ---------------------------------------------------------------------------
_KERNEL_CACHE = {}


def build_conv_kernel(cin, B, BLOCK, ncp, sched, groups):
    """sched: list of (block, goff) per tile; groups: list of (t0, nt, b).

    DRAM inputs (per core):
      xb    fp16 [B*BLOCK, cin]  full (padded) node features
      xs    fp16 [ncp, cin]      this core's dst-shard rows (padded)
      idxw  int16 [16, T, 8]     wrapped per-tile local src indices
      dl    fp32 [128, T]        per-slot dst offset within window (0..W-1)
      nm    fp32 [128, T]        per-slot edge norm (0 for padding)
      vdeg  fp32 [128, NC128]    1/deg for shard nodes (wrapped by chunk)
      wmat  fp32 [cin, 128]      weight
      iota  fp16 [128, 128]      col j = j in every partition
      pidx  fp32 [128, 1]        partition index column
    DRAM output:
      outT  fp32 [128, ncp]      (= (y @ W).T for this core's shard)
    """
    T = len(sched)
    key = (cin, B, BLOCK, ncp, hash(tuple(sched)))
    if key in _KERNEL_CACHE:
        return _KERNEL_CACHE[key]

    KC = cin // 128
    NC128 = ncp // 128
    # fp16 x staging: halves gather bytes (256B rows measured faster than
    # 512B fp32 on HW), deletes the cast stage; gathered values were being
    # cast to fp16 for the matmul anyway.
    half = True
    XDT = FH if half else FP

    nc = bacc.Bacc("TRN2", target_bir_lowering=False, debug=False,
                   num_devices=NCORES, num_swdge_queues=NQ,
                   dynamic_dma_scratch_size=16384)

    xb = nc.dram_tensor("xb", [B * BLOCK, cin], XDT, kind="ExternalInput").ap()
    xs = nc.dram_tensor("xs", [ncp, cin], XDT, kind="ExternalInput").ap()
    Tm = max(T, 1)  # zero-size DRAM tensors are awkward; keep >=1
    idxw = nc.dram_tensor("idxw", [16, Tm, 8], mybir.dt.int16,
                          kind="ExternalInput").ap()
    dl = nc.dram_tensor("dl", [128, Tm], FP, kind="ExternalInput").ap()
    nm = nc.dram_tensor("nm", [128, Tm], FP, kind="ExternalInput").ap()
    vdeg = nc.dram_tensor("vdeg", [128, NC128], FP, kind="ExternalInput").ap()
    wmat = nc.dram_tensor("wmat", [cin, 128], FP, kind="ExternalInput").ap()
    iota = nc.dram_tensor("iota", [128, 128], FH, kind="ExternalInput").ap()
    iota3 = nc.dram_tensor("iota3", [128, TG * W], FP,
                           kind="ExternalInput").ap()
    zro = nc.dram_tensor("zro", [128, GSPAN], FH, kind="ExternalInput").ap()
    pidx = nc.dram_tensor("pidx", [128, 1], FP, kind="ExternalInput").ap()
    outT = nc.dram_tensor("outT", [128, ncp], FP, kind="ExternalOutput").ap()

    with tile.TileContext(nc) as tc:
        with (
            tc.tile_pool(name="const", bufs=1) as constp,
            tc.tile_pool(name="y", bufs=1) as yp,
            tc.tile_pool(name="meta", bufs=2) as metap,
            tc.tile_pool(name="xsl", bufs=3) as xslp,
            tc.tile_pool(name="gath", bufs=5 if cin == 128 else 4) as gathp,
            tc.tile_pool(name="ghalf", bufs=3) as ghp,
            tc.tile_pool(name="nh", bufs=4) as nhp,
            tc.tile_pool(name="eq", bufs=3) as eqp,
            tc.tile_pool(name="pg", bufs=6, space="PSUM") as pgp,
            tc.tile_pool(name="fin", bufs=1, space="PSUM") as finp,
            tc.tile_pool(name="ob", bufs=2) as obp,
        ):
            iota_t = constp.tile([128, 128], FH)
            nc.sync.dma_start(iota_t[:], iota[:])
            iota3_t = constp.tile([128, TG, W], FP)
            nc.sync.dma_start(iota3_t[:], iota3[:])
            zro_t = constp.tile([128, GSPAN], FH)
            nc.sync.dma_start(zro_t[:], zro[:])
            pidx_t = constp.tile([128, 1], FP)
            nc.sync.dma_start(pidx_t[:], pidx[:])
            vdeg_t = constp.tile([128, NC128], FP)
            nc.sync.dma_start(vdeg_t[:], vdeg[:])
            w_t = []
            for kc in range(KC):
                wt = constp.tile([128, 128], FP, name=f"wt{kc}")
                nc.sync.dma_start(wt[:], wmat[kc * 128:(kc + 1) * 128, :])
                w_t.append(wt)
            y_t = [yp.tile([128, ncp], FP, name=f"yt{kc}") for kc in range(KC)]

            regs = {}
            for nt in sorted({g[1] for g in groups}):
                regs[nt] = nc.gpsimd.to_reg(nt * 128)

            # ---- self-loop pass: y[:, chunk] = (xs_chunk^T) @ diag(1/deg)
            # (initializes y; pad chunks have vdeg=0 so y pad cols are 0)
            for k in range(NC128):
                xst = xslp.tile([128, cin], XDT)
                nc.sync.dma_start(xst[:], xs[k * 128:(k + 1) * 128, :])
                if half:
                    xsh = xst
                else:
                    xsh = xslp.tile([128, cin], FH)
                    nc.scalar.copy(xsh[:], xst[:])
                Dt = nhp.tile([128, 128], FH)
                nc.vector.tensor_scalar(
                    Dt[:], iota_t[:], pidx_t[:, 0:1], vdeg_t[:, k:k + 1],
                    mybir.AluOpType.is_equal, mybir.AluOpType.mult)
                for kc in range(KC):
                    sf_t = finp.tile([128, 128], mybir.dt.float32,
                                     name="sf_t")
                    nc.tensor.matmul(sf_t[:], xsh[:, kc * 128:(kc + 1) * 128],
                                     Dt[:], start=True, stop=True)
                    nc.scalar.copy(y_t[kc][:, k * 128:(k + 1) * 128], sf_t[:])

            # ---- edge tiles (gather + batched one-hot + group-psum accum)
            nstripe = (T + SM - 1) // SM if T > 0 else 0
            stripe_tiles = {}

            def load_stripe(s):
                st0 = s * SM
                stn = min(SM, T - st0)
                i_sb = metap.tile([128, SM, 8], mybir.dt.int16)
                for k in range(8):
                    nc.sync.dma_start(i_sb[16 * k:16 * (k + 1), 0:stn, :],
                                      idxw[:, st0:st0 + stn, :])
                d_sb = metap.tile([128, SM], FP)
                nc.sync.dma_start(d_sb[:, 0:stn], dl[:, st0:st0 + stn])
                n_sb = metap.tile([128, SM], FP)
                nc.sync.dma_start(n_sb[:, 0:stn], nm[:, st0:st0 + stn])
                stripe_tiles[s] = (i_sb, d_sb, n_sb)

            if nstripe > 0:
                load_stripe(0)
            for gi, (t0, nt, b) in enumerate(groups):
                s0 = t0 // SM
                if s0 + 1 < nstripe and s0 + 1 not in stripe_tiles \
                        and t0 - s0 * SM >= SM // 2:
                    load_stripe(s0 + 1)
                idx_sb, dl_sb, nm_sb = stripe_tiles[s0]
                lo = t0 - s0 * SM
                gbase = sched[t0][1]
                gw = sched[t0 + nt - 1][1] + W - gbase
                gt = gathp.tile([128, TG, cin], XDT)
                nc.gpsimd.dma_gather(
                    gt[:, 0:nt, :], xb[b * BLOCK:(b + 1) * BLOCK, :],
                    idx_sb[:, lo:lo + nt, :], nt * 128, regs[nt], cin,
                    queue_num=gi % NQ)
                if half:
                    ght = gt
                else:
                    ght = ghp.tile([128, TG, cin], FH)
                    nc.scalar.copy(ght[:, 0:nt, :], gt[:, 0:nt, :])
                eq_t = eqp.tile([128, TG, W], FP)
                dlb = dl_sb[:, lo:lo + nt].unsqueeze(2).broadcast_to(
                    (128, nt, W))
                nmb = nm_sb[:, lo:lo + nt].unsqueeze(2).broadcast_to(
                    (128, nt, W))
                nc.vector.tensor_tensor(eq_t[:, 0:nt, :], iota3_t[:, 0:nt, :],
                                        dlb, mybir.AluOpType.is_equal)
                nh_g = nhp.tile([128, TG, W], FH)
                nc.vector.tensor_tensor(nh_g[:, 0:nt, :], eq_t[:, 0:nt, :],
                                        nmb, mybir.AluOpType.mult)
                pg_t = [pgp.tile([128, GSPAN], mybir.dt.float32, name="pg")
                        for kc in range(KC)]
                for kc in range(KC):
                    nc.tensor.matmul(pg_t[kc][:, 0:gw], iota_t[:],
                                     zro_t[:, 0:gw], start=True, stop=False,
                                     skip_group_check=True)
                for jt in range(nt):
                    po = sched[t0 + jt][1] - gbase
                    for kc in range(KC):
                        nc.tensor.matmul(
                            pg_t[kc][:, po:po + W],
                            ght[:, jt, kc * 128:(kc + 1) * 128],
                            nh_g[:, jt, :], start=False, stop=(jt == nt - 1),
                            skip_group_check=True)
                for kc in range(KC):
                    nc.vector.tensor_add(
                        y_t[kc][:, gbase:gbase + gw],
                        y_t[kc][:, gbase:gbase + gw], pg_t[kc][:, 0:gw])

            # ---- final: outT = sum_kc W[kc].T @ y[kc]
            FC = 512
            q0 = 0
            while q0 < ncp:
                fq = min(FC, ncp - q0)
                fp_t = finp.tile([128, fq], mybir.dt.float32)
                for kc in range(KC):
                    nc.tensor.matmul(fp_t[:], w_t[kc][:],
                                     y_t[kc][:, q0:q0 + fq],
                                     start=(kc == 0), stop=(kc == KC - 1))
                ob_t = obp.tile([128, fq], FP)
                nc.scalar.copy(ob_t[:], fp_t[:])
                nc.sync.dma_start(outT[:, q0:q0 + fq], ob_t[:])
                q0 += fq

    nc.compile()
    _KERNEL_CACHE[key] = nc
    return nc


# ---------------------------------------------------------------------------
# Host-side metadata build for one graph level
# ---------------------------------------------------------------------------
def build_level_meta(src, dst, n):
    """src/dst: active edges (compacted, relabeled) int64 arrays; n nodes.

    Builds the SPMD-shared joint-greedy tile schedule + per-core slot data.
    """
    n_core = (n + NCORES - 1) // NCORES
    ncp = max(128, ((n_core + 127) // 128) * 128)
    B = max(1, (n + IDXMAX - 1) // IDXMAX)
    BLOCK = ((n + B - 1) // B + 7) // 8 * 8
    assert BLOCK <= IDXMAX

    deg = (np.bincount(dst, minlength=n) + 1.0).astype(NPF)
    dinv = (1.0 / np.sqrt(deg)).astype(NPF)
    enorm = (dinv[src] * dinv[dst]).astype(NPF)

    core = dst // n_core
    dloc = (dst - core * n_core).astype(np.int64)
    blk = src // BLOCK

    order = np.lexsort((dloc, blk, core))
    sc = src[order]
    dc = dloc[order]
    wc = enorm[order]
    keys = (core[order] * B + blk[order]).astype(np.int64)

    # segment bounds per (core, block)
    bounds = np.searchsorted(keys, np.arange(NCORES * B + 1))

    sched = []          # (block, goff)
    tile_take = []      # per tile: list of (core, pos, cnt)
    for b in range(B):
        pos = [int(bounds[c * B + b]) for c in range(NCORES)]
        hi = [int(bounds[c * B + b + 1]) for c in range(NCORES)]
        while True:
            base = None
            for c in range(NCORES):
                if pos[c] < hi[c]:
                    v = int(dc[pos[c]])
                    if base is None or v < base:
                        base = v
            if base is None:
                break
            base = min(base, ncp - W)
            takes = []
            for c in range(NCORES):
                if pos[c] < hi[c]:
                    e = pos[c] + int(np.searchsorted(
                        dc[pos[c]:hi[c]], base + W, side="left"))
                    cnt = min(128, e - pos[c])
                    if cnt > 0:
                        takes.append((c, pos[c], cnt))
                        pos[c] += cnt
            sched.append((b, base))
            tile_take.append(takes)

    T = len(sched)
    Tm = max(T, 1)
    idx16 = np.zeros((NCORES, Tm, 128), dtype=np.int16)
    dl_a = np.zeros((NCORES, Tm, 128), dtype=NPF)
    nm_a = np.zeros((NCORES, Tm, 128), dtype=NPF)
    for t, takes in enumerate(tile_take):
        bb, goff = sched[t]
        for c, p0, cnt in takes:
            sl = slice(p0, p0 + cnt)
            idx16[c, t, :cnt] = (sc[sl] - bb * BLOCK).astype(np.int16)
            dl_a[c, t, :cnt] = (dc[sl] - goff).astype(NPF)
            nm_a[c, t, :cnt] = wc[sl].astype(NPF)

    # groups: runs of <=TG tiles, same block, within one stripe, and with
    # dst-window span <= GSPAN (one PSUM bank accumulates the whole group)
    groups = []
    t = 0
    while t < T:
        b = sched[t][0]
        bend = t
        while bend < T and sched[bend][0] == b:
            bend += 1
        while t < bend:
            nt = min(TG, bend - t, (t // SM + 1) * SM - t)
            while nt > 1 and sched[t + nt - 1][1] - sched[t][1] > GSPAN - W:
                nt -= 1
            groups.append((t, nt, b))
            t += nt

    NC128 = ncp // 128
    per_core = []
    for c in range(NCORES):
        idxw = idx16[c].reshape(Tm, 8, 16).transpose(2, 0, 1).copy()
        dlw = dl_a[c].transpose(1, 0).copy()
        nmw = nm_a[c].transpose(1, 0).copy()
        vd = np.zeros((128, NC128), dtype=NPF)
        lo = c * n_core
        cnt = max(0, min(n - lo, n_core))
        if cnt > 0:
            col = np.zeros(NC128 * 128, dtype=NPF)
            col[:cnt] = 1.0 / deg[lo:lo + cnt]
            vd[:, :] = col.reshape(NC128, 128).T
        per_core.append({"idxw": idxw, "dl": dlw, "nm": nmw, "vdeg": vd})

    return {
        "per_core": per_core, "n": n, "n_core": n_core, "ncp": ncp,
        "B": B, "BLOCK": BLOCK, "T": T, "sched": tuple(sched),
        "groups": groups, "deg": deg,
    }


def run_conv(meta, x_full, Wmat):
    """x_full: [n, cin] fp32 (full, unpadded); Wmat: [cin, 128].
    Returns y_out [n, 128] fp32 = GCN aggregation @ W (no bias)."""
    cin = x_full.shape[1]
    nc = build_conv_kernel(cin, meta["B"], meta["BLOCK"], meta["ncp"],
                           meta["sched"], meta["groups"])
    n, n_core, ncp = meta["n"], meta["n_core"], meta["ncp"]
    xb = np.zeros((meta["B"] * meta["BLOCK"], cin), dtype=NPH)
    xb[:n] = x_full
    Wf = np.ascontiguousarray(Wmat.astype(NPF))
    in_maps = []
    for c in range(NCORES):
        pc = meta["per_core"][c]
        lo = c * n_core
        xsc = np.zeros((ncp, cin), dtype=NPH)
        cnt = max(0, min(n - lo, n_core))
        if cnt > 0:
            xsc[:cnt] = x_full[lo:lo + cnt]
        in_maps.append({
            "xb": xb, "xs": xsc, "idxw": pc["idxw"], "dl": pc["dl"],
            "nm": pc["nm"], "vdeg": pc["vdeg"], "wmat": Wf,
            "iota": IOTA, "iota3": IOTA3, "zro": ZROS, "pidx": PIDX,
        })
    trace = bool(int(os.environ.get("GNN_TRACE", "0")))
    res = bass_utils.run_bass_kernel_spmd(
        nc, in_maps, core_ids=list(range(NCORES)), trace=trace)
    if res.exec_time_ns is not None:
        EXEC_NS.append(res.exec_time_ns)
    outs = [r["outT"] for r in res.results]
    y = np.concatenate([o.T for o in outs], axis=0)  # [8*ncp, 128]
    if ncp != n_core:
        y = y.reshape(NCORES, ncp, 128)[:, :n_core].reshape(-1, 128)
    return np.ascontiguousarray(y[:n])


# ---------------------------------------------------------------------------
# Host reference pieces (numpy, matching reference.py semantics)
# ---------------------------------------------------------------------------
def bn_relu(x, g, beta):
    m = x.mean(axis=0, dtype=np.float64).astype(NPF)
    v = ((x - m) ** 2).mean(axis=0, dtype=np.float64).astype(NPF)
    out = (x - m) * (1.0 / np.sqrt(v + EPS)) * g + beta
    return np.maximum(out, 0.0).astype(NPF)


def topk_host(score, k):
    # match jax.lax.top_k: descending values, ties -> lower index first
    idx = np.argsort(-score, kind="stable")[:k]
    return idx.astype(np.int64)


def kernel(x, edge_index, in_W, in_b, dn_W, dn_b, dn_g, dn_beta, pool_w,
           bot_W, bot_b, up_W, up_b, up_g, up_beta, out_W, out_b):
    x = np.asarray(x, dtype=NPF)
    src = np.asarray(edge_index[0], dtype=np.int64)
    dst = np.asarray(edge_index[1], dtype=np.int64)
    n = x.shape[0]

    meta0 = build_level_meta(src, dst, n)

    # in conv
    x = run_conv(meta0, x, np.asarray(in_W)) + np.asarray(in_b, dtype=NPF)

    xs, stack = [], []
    cur_src, cur_dst, cur_n, cur_meta = src, dst, n, meta0
    for i in range(DEPTH):
        x = run_conv(cur_meta, x, np.asarray(dn_W[i])) + \
            np.asarray(dn_b[i], dtype=NPF)
        x = bn_relu(x, np.asarray(dn_g[i], dtype=NPF),
                    np.asarray(dn_beta[i], dtype=NPF))
        xs.append(x)
        k = int(RATIO * cur_n)
        w = np.asarray(pool_w[i], dtype=NPF)
        score = np.tanh(x @ w / np.sqrt((w * w).sum()))
        idx = topk_host(score, k)
        new_id = np.zeros(cur_n, dtype=np.int64)
        new_id[idx] = np.arange(k)
        kept = np.zeros(cur_n, dtype=bool)
        kept[idx] = True
        emask = kept[cur_src] & kept[cur_dst]
        stack.append((cur_meta, idx, cur_n))
        cur_src = new_id[cur_src[emask]]
        cur_dst = new_id[cur_dst[emask]]
        cur_n = k
        x = x[idx]
        cur_meta = build_level_meta(cur_src, cur_dst, cur_n)

    # bottleneck
    x = run_conv(cur_meta, x, np.asarray(bot_W)) + \
        np.asarray(bot_b, dtype=NPF)
    x = np.maximum(x, 0.0)

    for i in range(DEPTH):
        p_meta, idx, pn = stack[DEPTH - 1 - i]
        xf = np.zeros((pn, x.shape[1]), dtype=NPF)
        xf[idx] = x
        x = np.concatenate([xf, xs[DEPTH - 1 - i]], axis=1)
        x = run_conv(p_meta, x, np.asarray(up_W[i])) + \
            np.asarray(up_b[i], dtype=NPF)
        x = bn_relu(x, np.asarray(up_g[i], dtype=NPF),
                    np.asarray(up_beta[i], dtype=NPF))
        cur_meta = p_meta

    out = run_conv(cur_meta, x, np.asarray(out_W)) + \
        np.asarray(out_b, dtype=NPF)
    return out.astype(np.float32)


# revision 19
# speedup vs baseline: 5.9218x; 1.0119x over previous
"""GraphUNet Trainium kernel (v2).

Architecture: 9 GCN convs (in, dn0, dn1, dn2, bottom, up0, up1, up2, out)
with top-k pooling / unpooling and batch-norm+relu between convs.

Device per conv: edge aggregation y[d] = sum_{e: dst=d} norm_e * x[src_e]
 + self-loop x[d]/deg[d], then out = (y @ W).T via:
  - dma_gather of x rows (fp16, 4 SWDGE queues round-robin) in tiles of
    128 edges packed by a joint-greedy sliding-window schedule (shared
    across the 8 SPMD cores; per-core slot data fills the tiles),
  - fp16 one-hot matmul: psum[128c x 64] += x_rows^T @ onehot(dst)*norm,
  - self-loop term via sequential shard stream + diagonal matmul
    (no gather descriptors; also serves as the y initializer),
  - final fp32 matmul streaming y through W.

Host (numpy): sharding/metadata build, top-k pools, edge relabeling,
degree/norm precompute, batch-norm, relu, bias, concat.

Sharding: dst-node ranges across 8 cores; x replicated to all cores
(graph/data parallel; halo exchange realized as full replication of the
per-conv feature table, re-staged by the host between launches).
"""

import math
import os
import sys

import numpy as np

sys.path.insert(0, "/opt/trn_rl_repo")

import concourse.bass as bass  # noqa: E402,F401
import concourse.bacc as bacc  # noqa: E402
import concourse.tile as tile  # noqa: E402
from concourse import mybir  # noqa: E402
from concourse import bass_utils  # noqa: E402

# ---- problem constants (hardcoded per task statement) ----
C_IN = 128
H = 128
DEPTH = 3
RATIO = 0.5
EPS = 1e-5
NCORES = 8
W = 64            # one-hot dst window width
TG = 8            # tiles per gather call (1024 idxs = SWDGE ring limit)
SM = 768          # tiles per metadata stripe (multiple of TG)
NQ = 4            # SWDGE queues (desc-gen parallelism)
IDXMAX = 32768    # int16 gather index reach

FP = mybir.dt.float32
FH = mybir.dt.float16
NPF = np.float32
NPH = np.float16

IOTA = np.broadcast_to(np.arange(128, dtype=NPH), (128, 128)).copy()
PIDX = np.arange(128, dtype=NPF).reshape(128, 1).copy()
IOTA3 = np.broadcast_to(np.arange(W, dtype=NPF), (128, TG, W)).reshape(
    128, TG * W).copy()
ZROS = np.zeros((128, 512), dtype=NPH)
GSPAN = 512       # group psum window width (one PSUM bank)

EXEC_NS = []  # accumulated HW exec times when tracing enabled


# ---------------------------------------------------------------------------
# Bass kernel builder (one conv shape + baked tile schedule).
# ---------------------------------------------------------------------------
_KERNEL_CACHE = {}


def build_conv_kernel(cin, B, BLOCK, ncp, sched, groups):
    """sched: list of (block, goff) per tile; groups: list of (t0, nt, b).

    DRAM inputs (per core):
      xb    fp16 [B*BLOCK, cin]  full (padded) node features
      xs    fp16 [ncp, cin]      this core's dst-shard rows (padded)
      idxw  int16 [16, T, 8]     wrapped per-tile local src indices
      dl    fp32 [128, T]        per-slot dst offset within window (0..W-1)
      nm    fp32 [128, T]        per-slot edge norm (0 for padding)
      vdeg  fp32 [128, NC128]    1/deg for shard nodes (wrapped by chunk)
      wmat  fp16 [cin, 128]      weight
      iota  fp16 [128, 128]      col j = j in every partition
      pidx  fp32 [128, 1]        partition index column
    DRAM output:
      outT  fp32 [128, ncp]      (= (y @ W).T for this core's shard)
    """
    T = len(sched)
    key = (cin, B, BLOCK, ncp, hash(tuple(sched)))
    if key in _KERNEL_CACHE:
        return _KERNEL_CACHE[key]

    KC = cin // 128
    NC128 = ncp // 128
    # fp16 x staging: halves gather bytes (256B rows measured faster than
    # 512B fp32 on HW), deletes the cast stage; gathered values were being
    # cast to fp16 for the matmul anyway.
    half = True
    XDT = FH if half else FP

    nc = bacc.Bacc("TRN2", target_bir_lowering=False, debug=False,
                   num_devices=NCORES, num_swdge_queues=NQ,
                   dynamic_dma_scratch_size=16384)

    xb = nc.dram_tensor("xb", [B * BLOCK, cin], XDT, kind="ExternalInput").ap()
    xs = nc.dram_tensor("xs", [ncp, cin], XDT, kind="ExternalInput").ap()
    Tm = max(T, 1)  # zero-size DRAM tensors are awkward; keep >=1
    idxw = nc.dram_tensor("idxw", [16, Tm, 8], mybir.dt.int16,
                          kind="ExternalInput").ap()
    dl = nc.dram_tensor("dl", [128, Tm], FP, kind="ExternalInput").ap()
    nm = nc.dram_tensor("nm", [128, Tm], FP, kind="ExternalInput").ap()
    vdeg = nc.dram_tensor("vdeg", [128, NC128], FP, kind="ExternalInput").ap()
    wmat = nc.dram_tensor("wmat", [cin, 128], FH, kind="ExternalInput").ap()
    iota = nc.dram_tensor("iota", [128, 128], FH, kind="ExternalInput").ap()
    iota3 = nc.dram_tensor("iota3", [128, TG * W], FP,
                           kind="ExternalInput").ap()
    zro = nc.dram_tensor("zro", [128, GSPAN], FH, kind="ExternalInput").ap()
    pidx = nc.dram_tensor("pidx", [128, 1], FP, kind="ExternalInput").ap()
    outT = nc.dram_tensor("outT", [128, ncp], FP, kind="ExternalOutput").ap()

    with tile.TileContext(nc) as tc:
        with (
            tc.tile_pool(name="const", bufs=1) as constp,
            tc.tile_pool(name="y", bufs=1) as yp,
            tc.tile_pool(name="meta", bufs=2) as metap,
            tc.tile_pool(name="xsl", bufs=3) as xslp,
            tc.tile_pool(name="gath", bufs=5 if cin == 128 else 6) as gathp,
            tc.tile_pool(name="ghalf", bufs=3) as ghp,
            tc.tile_pool(name="nh", bufs=4) as nhp,
            tc.tile_pool(name="eq", bufs=3) as eqp,
            tc.tile_pool(name="pg", bufs=6, space="PSUM") as pgp,
            tc.tile_pool(name="fin", bufs=1, space="PSUM") as finp,
            tc.tile_pool(name="ob", bufs=2) as obp,
        ):
            iota_t = constp.tile([128, 128], FH)
            nc.sync.dma_start(iota_t[:], iota[:])
            iota3_t = constp.tile([128, TG, W], FP)
            nc.sync.dma_start(iota3_t[:], iota3[:])
            zro_t = constp.tile([128, GSPAN], FH)
            nc.sync.dma_start(zro_t[:], zro[:])
            pidx_t = constp.tile([128, 1], FP)
            nc.sync.dma_start(pidx_t[:], pidx[:])
            vdeg_t = constp.tile([128, NC128], FP)
            nc.sync.dma_start(vdeg_t[:], vdeg[:])
            w_t = []
            for kc in range(KC):
                wt = constp.tile([128, 128], FH, name=f"wt{kc}")
                nc.sync.dma_start(wt[:], wmat[kc * 128:(kc + 1) * 128, :])
                w_t.append(wt)
            y_t = [yp.tile([128, ncp], FP, name=f"yt{kc}") for kc in range(KC)]

            regs = {}
            for nt in sorted({g[1] for g in groups}):
                regs[nt] = nc.gpsimd.to_reg(nt * 128)

            # ---- self-loop pass: y[:, chunk] = (xs_chunk^T) @ diag(1/deg)
            # (initializes y; pad chunks have vdeg=0 so y pad cols are 0)
            for k in range(NC128):
                xst = xslp.tile([128, cin], XDT)
                nc.sync.dma_start(xst[:], xs[k * 128:(k + 1) * 128, :])
                if half:
                    xsh = xst
                else:
                    xsh = xslp.tile([128, cin], FH)
                    nc.scalar.copy(xsh[:], xst[:])
                Dt = nhp.tile([128, 128], FH)
                nc.vector.tensor_scalar(
                    Dt[:], iota_t[:], pidx_t[:, 0:1], vdeg_t[:, k:k + 1],
                    mybir.AluOpType.is_equal, mybir.AluOpType.mult)
                for kc in range(KC):
                    sf_t = finp.tile([128, 128], mybir.dt.float32,
                                     name="sf_t")
                    nc.tensor.matmul(sf_t[:], xsh[:, kc * 128:(kc + 1) * 128],
                                     Dt[:], start=True, stop=True)
                    nc.scalar.copy(y_t[kc][:, k * 128:(k + 1) * 128], sf_t[:])

            # ---- edge tiles (gather + batched one-hot + group-psum accum)
            nstripe = (T + SM - 1) // SM if T > 0 else 0
            stripe_tiles = {}

            def load_stripe(s):
                st0 = s * SM
                stn = min(SM, T - st0)
                i_sb = metap.tile([128, SM, 8], mybir.dt.int16)
                for k in range(8):
                    nc.sync.dma_start(i_sb[16 * k:16 * (k + 1), 0:stn, :],
                                      idxw[:, st0:st0 + stn, :])
                d_sb = metap.tile([128, SM], FP)
                nc.sync.dma_start(d_sb[:, 0:stn], dl[:, st0:st0 + stn])
                n_sb = metap.tile([128, SM], FP)
                nc.sync.dma_start(n_sb[:, 0:stn], nm[:, st0:st0 + stn])
                stripe_tiles[s] = (i_sb, d_sb, n_sb)

            if nstripe > 0:
                load_stripe(0)
            for gi, (t0, nt, b) in enumerate(groups):
                s0 = t0 // SM
                if s0 + 1 < nstripe and s0 + 1 not in stripe_tiles \
                        and t0 - s0 * SM >= SM // 2:
                    load_stripe(s0 + 1)
                idx_sb, dl_sb, nm_sb = stripe_tiles[s0]
                lo = t0 - s0 * SM
                gbase = sched[t0][1]
                gw = sched[t0 + nt - 1][1] + W - gbase
                gt = gathp.tile([128, TG, cin], XDT)
                nc.gpsimd.dma_gather(
                    gt[:, 0:nt, :], xb[b * BLOCK:(b + 1) * BLOCK, :],
                    idx_sb[:, lo:lo + nt, :], nt * 128, regs[nt], cin,
                    queue_num=gi % NQ)
                if half:
                    ght = gt
                else:
                    ght = ghp.tile([128, TG, cin], FH)
                    nc.scalar.copy(ght[:, 0:nt, :], gt[:, 0:nt, :])
                eq_t = eqp.tile([128, TG, W], FP)
                dlb = dl_sb[:, lo:lo + nt].unsqueeze(2).broadcast_to(
                    (128, nt, W))
                nmb = nm_sb[:, lo:lo + nt].unsqueeze(2).broadcast_to(
                    (128, nt, W))
                nc.vector.tensor_tensor(eq_t[:, 0:nt, :], iota3_t[:, 0:nt, :],
                                        dlb, mybir.AluOpType.is_equal)
                nh_g = nhp.tile([128, TG, W], FH)
                nc.vector.tensor_tensor(nh_g[:, 0:nt, :], eq_t[:, 0:nt, :],
                                        nmb, mybir.AluOpType.mult)
                pg_t = [pgp.tile([128, GSPAN], mybir.dt.float32, name="pg")
                        for kc in range(KC)]
                for kc in range(KC):
                    nc.tensor.matmul(pg_t[kc][:, 0:gw], iota_t[:],
                                     zro_t[:, 0:gw], start=True, stop=False,
                                     skip_group_check=True)
                for jt in range(nt):
                    po = sched[t0 + jt][1] - gbase
                    for kc in range(KC):
                        nc.tensor.matmul(
                            pg_t[kc][:, po:po + W],
                            ght[:, jt, kc * 128:(kc + 1) * 128],
                            nh_g[:, jt, :], start=False, stop=(jt == nt - 1),
                            skip_group_check=True)
                for kc in range(KC):
                    nc.vector.tensor_add(
                        y_t[kc][:, gbase:gbase + gw],
                        y_t[kc][:, gbase:gbase + gw], pg_t[kc][:, 0:gw])

            # ---- final: outT = sum_kc W[kc].T @ y[kc]
            FC = 512
            q0 = 0
            while q0 < ncp:
                fq = min(FC, ncp - q0)
                fp_t = finp.tile([128, fq], mybir.dt.float32)
                for kc in range(KC):
                    yh_t = obp.tile([128, fq], FH, name="yh")
                    nc.vector.tensor_copy(yh_t[:], y_t[kc][:, q0:q0 + fq])
                    nc.tensor.matmul(fp_t[:], w_t[kc][:], yh_t[:],
                                     start=(kc == 0), stop=(kc == KC - 1))
                ob_t = obp.tile([128, fq], FP)
                nc.scalar.copy(ob_t[:], fp_t[:])
                nc.sync.dma_start(outT[:, q0:q0 + fq], ob_t[:])
                q0 += fq

    nc.compile()
    _KERNEL_CACHE[key] = nc
    return nc


# ---------------------------------------------------------------------------
# Host-side metadata build for one graph level
# ---------------------------------------------------------------------------
def build_level_meta(src, dst, n):
    """src/dst: active edges (compacted, relabeled) int64 arrays; n nodes.

    Builds the SPMD-shared joint-greedy tile schedule + per-core slot data.
    """
    n_core = (n + NCORES - 1) // NCORES
    ncp = max(128, ((n_core + 127) // 128) * 128)
    B = max(1, (n + IDXMAX - 1) // IDXMAX)
    BLOCK = ((n + B - 1) // B + 7) // 8 * 8
    assert BLOCK <= IDXMAX

    deg = (np.bincount(dst, minlength=n) + 1.0).astype(NPF)
    dinv = (1.0 / np.sqrt(deg)).astype(NPF)
    enorm = (dinv[src] * dinv[dst]).astype(NPF)

    core = dst // n_core
    dloc = (dst - core * n_core).astype(np.int64)
    blk = src // BLOCK

    order = np.lexsort((dloc, blk, core))
    sc = src[order]
    dc = dloc[order]
    wc = enorm[order]
    keys = (core[order] * B + blk[order]).astype(np.int64)

    # segment bounds per (core, block)
    bounds = np.searchsorted(keys, np.arange(NCORES * B + 1))

    sched = []          # (block, goff)
    tile_take = []      # per tile: list of (core, pos, cnt)
    for b in range(B):
        pos = [int(bounds[c * B + b]) for c in range(NCORES)]
        hi = [int(bounds[c * B + b + 1]) for c in range(NCORES)]
        while True:
            base = None
            for c in range(NCORES):
                if pos[c] < hi[c]:
                    v = int(dc[pos[c]])
                    if base is None or v < base:
                        base = v
            if base is None:
                break
            base = min(base, ncp - W)
            takes = []
            for c in range(NCORES):
                if pos[c] < hi[c]:
                    e = pos[c] + int(np.searchsorted(
                        dc[pos[c]:hi[c]], base + W, side="left"))
                    cnt = min(128, e - pos[c])
                    if cnt > 0:
                        takes.append((c, pos[c], cnt))
                        pos[c] += cnt
            sched.append((b, base))
            tile_take.append(takes)

    T = len(sched)
    Tm = max(T, 1)
    idx16 = np.zeros((NCORES, Tm, 128), dtype=np.int16)
    dl_a = np.zeros((NCORES, Tm, 128), dtype=NPF)
    nm_a = np.zeros((NCORES, Tm, 128), dtype=NPF)
    for t, takes in enumerate(tile_take):
        bb, goff = sched[t]
        for c, p0, cnt in takes:
            sl = slice(p0, p0 + cnt)
            idx16[c, t, :cnt] = (sc[sl] - bb * BLOCK).astype(np.int16)
            dl_a[c, t, :cnt] = (dc[sl] - goff).astype(NPF)
            nm_a[c, t, :cnt] = wc[sl].astype(NPF)

    # groups: runs of <=TG tiles, same block, within one stripe, and with
    # dst-window span <= GSPAN (one PSUM bank accumulates the whole group)
    groups = []
    t = 0
    while t < T:
        b = sched[t][0]
        bend = t
        while bend < T and sched[bend][0] == b:
            bend += 1
        while t < bend:
            nt = min(TG, bend - t, (t // SM + 1) * SM - t)
            while nt > 1 and sched[t + nt - 1][1] - sched[t][1] > GSPAN - W:
                nt -= 1
            groups.append((t, nt, b))
            t += nt

    NC128 = ncp // 128
    per_core = []
    for c in range(NCORES):
        idxw = idx16[c].reshape(Tm, 8, 16).transpose(2, 0, 1).copy()
        dlw = dl_a[c].transpose(1, 0).copy()
        nmw = nm_a[c].transpose(1, 0).copy()
        vd = np.zeros((128, NC128), dtype=NPF)
        lo = c * n_core
        cnt = max(0, min(n - lo, n_core))
        if cnt > 0:
            col = np.zeros(NC128 * 128, dtype=NPF)
            col[:cnt] = 1.0 / deg[lo:lo + cnt]
            vd[:, :] = col.reshape(NC128, 128).T
        per_core.append({"idxw": idxw, "dl": dlw, "nm": nmw, "vdeg": vd})

    return {
        "per_core": per_core, "n": n, "n_core": n_core, "ncp": ncp,
        "B": B, "BLOCK": BLOCK, "T": T, "sched": tuple(sched),
        "groups": groups, "deg": deg,
    }


def run_conv(meta, x_full, Wmat):
    """x_full: [n, cin] fp32 (full, unpadded); Wmat: [cin, 128].
    Returns y_out [n, 128] fp32 = GCN aggregation @ W (no bias)."""
    cin = x_full.shape[1]
    nc = build_conv_kernel(cin, meta["B"], meta["BLOCK"], meta["ncp"],
                           meta["sched"], meta["groups"])
    n, n_core, ncp = meta["n"], meta["n_core"], meta["ncp"]
    xb = np.zeros((meta["B"] * meta["BLOCK"], cin), dtype=NPH)
    xb[:n] = x_full
    Wf = np.ascontiguousarray(Wmat.astype(NPH))
    in_maps = []
    for c in range(NCORES):
        pc = meta["per_core"][c]
        lo = c * n_core
        xsc = np.zeros((ncp, cin), dtype=NPH)
        cnt = max(0, min(n - lo, n_core))
        if cnt > 0:
            xsc[:cnt] = x_full[lo:lo + cnt]
        in_maps.append({
            "xb": xb, "xs": xsc, "idxw": pc["idxw"], "dl": pc["dl"],
            "nm": pc["nm"], "vdeg": pc["vdeg"], "wmat": Wf,
            "iota": IOTA, "iota3": IOTA3, "zro": ZROS, "pidx": PIDX,
        })
    trace = bool(int(os.environ.get("GNN_TRACE", "0")))
    res = bass_utils.run_bass_kernel_spmd(
        nc, in_maps, core_ids=list(range(NCORES)), trace=trace)
    if res.exec_time_ns is not None:
        EXEC_NS.append(res.exec_time_ns)
    outs = [r["outT"] for r in res.results]
    y = np.concatenate([o.T for o in outs], axis=0)  # [8*ncp, 128]
    if ncp != n_core:
        y = y.reshape(NCORES, ncp, 128)[:, :n_core].reshape(-1, 128)
    return np.ascontiguousarray(y[:n])


# ---------------------------------------------------------------------------
# Host reference pieces (numpy, matching reference.py semantics)
# ---------------------------------------------------------------------------
def bn_relu(x, g, beta):
    m = x.mean(axis=0, dtype=np.float64).astype(NPF)
    v = ((x - m) ** 2).mean(axis=0, dtype=np.float64).astype(NPF)
    out = (x - m) * (1.0 / np.sqrt(v + EPS)) * g + beta
    return np.maximum(out, 0.0).astype(NPF)


def topk_host(score, k):
    # match jax.lax.top_k: descending values, ties -> lower index first
    idx = np.argsort(-score, kind="stable")[:k]
    return idx.astype(np.int64)


def kernel(x, edge_index, in_W, in_b, dn_W, dn_b, dn_g, dn_beta, pool_w,
           bot_W, bot_b, up_W, up_b, up_g, up_beta, out_W, out_b):
    x = np.asarray(x, dtype=NPF)
    src = np.asarray(edge_index[0], dtype=np.int64)
    dst = np.asarray(edge_index[1], dtype=np.int64)
    n = x.shape[0]

    meta0 = build_level_meta(src, dst, n)

    # in conv
    x = run_conv(meta0, x, np.asarray(in_W)) + np.asarray(in_b, dtype=NPF)

    xs, stack = [], []
    cur_src, cur_dst, cur_n, cur_meta = src, dst, n, meta0
    for i in range(DEPTH):
        x = run_conv(cur_meta, x, np.asarray(dn_W[i])) + \
            np.asarray(dn_b[i], dtype=NPF)
        x = bn_relu(x, np.asarray(dn_g[i], dtype=NPF),
                    np.asarray(dn_beta[i], dtype=NPF))
        xs.append(x)
        k = int(RATIO * cur_n)
        w = np.asarray(pool_w[i], dtype=NPF)
        score = np.tanh(x @ w / np.sqrt((w * w).sum()))
        idx = topk_host(score, k)
        new_id = np.zeros(cur_n, dtype=np.int64)
        new_id[idx] = np.arange(k)
        kept = np.zeros(cur_n, dtype=bool)
        kept[idx] = True
        emask = kept[cur_src] & kept[cur_dst]
        stack.append((cur_meta, idx, cur_n))
        cur_src = new_id[cur_src[emask]]
        cur_dst = new_id[cur_dst[emask]]
        cur_n = k
        x = x[idx]
        cur_meta = build_level_meta(cur_src, cur_dst, cur_n)

    # bottleneck
    x = run_conv(cur_meta, x, np.asarray(bot_W)) + \
        np.asarray(bot_b, dtype=NPF)
    x = np.maximum(x, 0.0)

    for i in range(DEPTH):
        p_meta, idx, pn = stack[DEPTH - 1 - i]
        xf = np.zeros((pn, x.shape[1]), dtype=NPF)
        xf[idx] = x
        x = np.concatenate([xf, xs[DEPTH - 1 - i]], axis=1)
        x = run_conv(p_meta, x, np.asarray(up_W[i])) + \
            np.asarray(up_b[i], dtype=NPF)
        x = bn_relu(x, np.asarray(up_g[i], dtype=NPF),
                    np.asarray(up_beta[i], dtype=NPF))
        cur_meta = p_meta

    out = run_conv(cur_meta, x, np.asarray(out_W)) + \
        np.asarray(out_b, dtype=NPF)
    return out.astype(np.float32)
